# revision 12
# baseline (speedup 1.0000x reference)
"""Trainium2 Bass kernel for nn_Affinity (graph-matching affinity matrix).

Math per sample (validated against the reference):
  out[(a,c),(b,c')] = sum_{e2,e1} G2[a,e2] H2[b,e2] Me[e2,e1] G1[c,e1] H1[c,e1]
                      + diag(vec(Mp))

Structural facts exploited:
  * G/H columns are one-hot, so the dense 1024x1024 output is a placement
    of Z[e1,(a,b)] = sum_e2 Me[e2,e1] P2[e2,(a,b)] at rows/cols given by
    graph-1 edge endpoints, plus diag(vec(Mp)).
  * Me = G1^T C1 G2 + G1^T C2 H2 + H1^T C2 G2 + H1^T C1 H2 with
    C_i = F1^T relu(l_i + l_i^T) F2  (32x32).  C1/C2 depend only on
    lambda/F, so all d=128 contractions run during the adjacency DMA
    window; the adjacency-dependent path only does 32-contractions.
  * P2[e2, 32a+b] = (32*head2(e2) + tail2(e2) == 32a+b): a single
    per-partition one-hot against an iota row, fed by one matmul.

Device (1 sample per NeuronCore, fully static instruction stream):
  1. Row-major edge ranks via masked prefix-scan; one-hot rank expansion;
     [G|H] recovered by accumulating matmuls against head/tail select
     constants GENERATED on device during the DMA window (no DMA).
  2. C-chain on PE/Act in parallel with the rank chain.
  3. Me^T = R^T G1 + S^T H1 with R|S = C1 G2 + C2 H2 | C2 G2 + C1 H2.
  4. Z = Me-contraction with P2; Z psum is DMA'd to HBM as f32 directly.
  5. aux psum [c | c' | MpT] is DMA'd f32 directly.  Host unshard is a
     pure placement (device-computed values at device-computed indices).
"""

import numpy as np

import concourse.bacc as bacc
import concourse.bass as bass
import concourse.mybir as mybir
import concourse.tile as tile
from concourse.bass_utils import run_bass_kernel_spmd

F32 = mybir.dt.float32
F16 = mybir.dt.float16
I32 = mybir.dt.int32
ALU = mybir.AluOpType
AX = mybir.AxisListType
AF = mybir.ActivationFunctionType

B, N, D, E = 8, 32, 128, 96
NCORES = 8


def build_program(debug: bool = False):
    nc = bacc.Bacc("TRN2", target_bir_lowering=False, debug=debug,
                   num_devices=NCORES)
    # lamA: lambda1 | lambda2 | A_src(128x8) | A_tgt(128x8), all f16-exact
    lamA = nc.dram_tensor("lamA", [128, 272], F16, kind="ExternalInput")
    # fu: F1 | F2 | U1 | U2  (each 128x32)
    fu = nc.dram_tensor("fu", [128, 128], F16, kind="ExternalInput")
    out_z = nc.dram_tensor("out_z", [96, 1024], F16, kind="ExternalOutput")
    out_aux = nc.dram_tensor("out_aux", [96, 34], F16, kind="ExternalOutput")

    with tile.TileContext(nc) as tc:
        with tc.tile_pool(name="sb", bufs=1) as sb, \
             tc.tile_pool(name="ps", bufs=1, space="PSUM") as ps:
            # ---- input DMAs (SP queue, HWDGE is serial across all queues)
            lam_sb = sb.tile([128, 272], F16, tag="lam_sb")
            nc.sync.dma_start(out=lam_sb[:], in_=lamA[:, :])
            fu_sb = sb.tile([128, 128], F16, tag="fu_sb")
            nc.sync.dma_start(out=fu_sb[:], in_=fu[:, :])
            l1_16, l2_16 = lam_sb[:, 0:128], lam_sb[:, 128:256]
            a16 = lam_sb[:, 256:272]            # A_src cols 0:8, A_tgt 8:16
            f1, f2 = fu_sb[:, 0:32], fu_sb[:, 32:64]
            u1, u2 = fu_sb[:, 64:96], fu_sb[:, 96:128]

            # ---- on-device constants (run during the input-DMA window) ----
            it32 = sb.tile([128, 129], I32, tag="it32")
            nc.gpsimd.iota(it32[:], pattern=[[1, 129]], base=0,
                           channel_multiplier=0)
            pi32 = sb.tile([128, 1], I32, tag="pi32")
            nc.gpsimd.iota(pi32[:], pattern=[[1, 1]], base=0,
                           channel_multiplier=1)
            i1k32 = sb.tile([96, 1024], I32, tag="i1k32")
            nc.gpsimd.iota(i1k32[:], pattern=[[1, 1024]], base=0,
                           channel_multiplier=0)
            io16 = sb.tile([128, 129], F16, tag="io16")
            nc.vector.tensor_copy(out=io16[:], in_=it32[:])
            i1k16 = sb.tile([96, 1024], F16, tag="i1k16")
            nc.vector.tensor_copy(out=i1k16[:], in_=i1k32[:])
            io32 = sb.tile([128, 128], F32, tag="io32")
            nc.vector.tensor_copy(out=io32[:], in_=it32[:, 0:128])
            pf32 = sb.tile([128, 1], F32, tag="pf32")
            nc.vector.tensor_copy(out=pf32[:], in_=pi32[:])
            pf16 = sb.tile([128, 1], F16, tag="pf16")
            nc.vector.tensor_copy(out=pf16[:], in_=pi32[:])
            # id16: PE-transpose identity; su32: strictly-lower ones
            id16 = sb.tile([128, 128], F16, tag="id16")
            nc.vector.tensor_scalar(out=id16[:], in0=io16[:, 0:128],
                                    scalar1=pf32[:, 0:1], scalar2=None,
                                    op0=ALU.is_equal)
            su32 = sb.tile([128, 128], F32, tag="su32")
            nc.vector.tensor_scalar(out=su32[:], in0=io32[:],
                                    scalar1=pf32[:, 0:1], scalar2=None,
                                    op0=ALU.is_gt)
            # head/tail select constants: head row = p//4,
            # tail col of slot (p,k) = 8*(p%4)+k
            q32 = sb.tile([128, 1], I32, tag="q32")
            nc.vector.tensor_scalar(out=q32[:], in0=pi32[:], scalar1=2,
                                    scalar2=None,
                                    op0=ALU.logical_shift_right)
            qf32 = sb.tile([128, 1], F32, tag="qf32")
            nc.vector.tensor_copy(out=qf32[:], in_=q32[:])
            headsel = sb.tile([128, 32], F16, tag="headsel")
            nc.vector.tensor_scalar(out=headsel[:], in0=io16[:, 0:32],
                                    scalar1=qf32[:, 0:1], scalar2=None,
                                    op0=ALU.is_equal)
            m32 = sb.tile([128, 1], I32, tag="m32")
            nc.vector.tensor_scalar(out=m32[:], in0=pi32[:], scalar1=3,
                                    scalar2=None, op0=ALU.bitwise_and)
            b832 = sb.tile([128, 1], I32, tag="b832")
            nc.vector.tensor_scalar(out=b832[:], in0=m32[:], scalar1=3,
                                    scalar2=None, op0=ALU.logical_shift_left)
            b8f = sb.tile([128, 1], F32, tag="b8f")
            nc.vector.tensor_copy(out=b8f[:], in_=b832[:])
            tailsel = sb.tile([128, 256], F16, tag="tailsel")
            for k in range(8):
                nc.vector.tensor_scalar(out=tailsel[:, 32 * k:32 * (k + 1)],
                                        in0=io16[:, 0:32],
                                        scalar1=b8f[:, 0:1],
                                        scalar2=float(k),
                                        op0=ALU.subtract, op1=ALU.is_equal)
            # matmul weight consts: wq = [32p | p | p | 0 | 0 | p]  (32,6)
            wq = sb.tile([32, 6], F16, tag="wq")
            nc.vector.memset(wq[:], 0.0)
            nc.vector.tensor_scalar(out=wq[:, 0:1], in0=pf16[0:32, :],
                                    scalar1=32.0, scalar2=None, op0=ALU.mult)
            nc.vector.tensor_copy(out=wq[:, 1:2], in_=pf16[0:32, :])
            nc.vector.tensor_copy(out=wq[:, 2:3], in_=pf16[0:32, :])
            nc.vector.tensor_copy(out=wq[:, 5:6], in_=pf16[0:32, :])

            # ================= rank chain (needs lamA) =================
            maskb = sb.tile([128, 16], F32, tag="maskb")
            nc.vector.tensor_copy(out=maskb[:], in_=a16)
            s2 = sb.tile([128, 2], F32, tag="s2")
            m3 = maskb[:].rearrange("p (g k) -> p g k", k=8)
            nc.vector.tensor_reduce(out=s2[:], in_=m3, axis=AX.X, op=ALU.add)
            pb = ps.tile([128, 2], F32, tag="psA", bufs=1)
            nc.tensor.matmul(out=pb[:], lhsT=su32[:], rhs=s2[:],
                             start=True, stop=True)
            r0 = sb.tile([128, 16], F32, tag="r0")
            r1h = sb.tile([128, 16], F32, tag="r1h")
            for g in (1, 0):
                nc.vector.tensor_tensor_scan(
                    out=r0[:, 8 * g:8 * (g + 1)],
                    data0=maskb[:, 8 * g:8 * (g + 1)],
                    data1=maskb[:, 8 * g:8 * (g + 1)],
                    initial=pb[:, g:g + 1],
                    op0=ALU.add, op1=ALU.bypass)
                nc.vector.tensor_tensor(out=r1h[:, 8 * g:8 * (g + 1)],
                                        in0=r0[:, 8 * g:8 * (g + 1)],
                                        in1=maskb[:, 8 * g:8 * (g + 1)],
                                        op=ALU.mult)

            # ---- one-hot rank expansion + [G|H] matmuls, graph2 then 1 ----
            oh2t = sb.tile([128, 768], F16, tag="oh2")
            oh1t = sb.tile([128, 768], F16, tag="oh1")
            gh2ps = ps.tile([32, 192], F32, tag="psD", bufs=1)
            gh1ps = ps.tile([32, 192], F32, tag="psC", bufs=1)
            gh2t = sb.tile([32, 192], F16, tag="gh2sb")
            gh1t = sb.tile([32, 192], F16, tag="gh1sb")
            oh = {1: oh2t, 0: oh1t}
            ghps = {1: gh2ps, 0: gh1ps}
            ghsb = {1: gh2t, 0: gh1t}
            for g in (1, 0):
                ohg = oh[g]
                for k in range(8):
                    nc.vector.tensor_scalar(
                        out=ohg[:, 96 * k:96 * (k + 1)],
                        in0=io16[:, 1:97],
                        scalar1=r1h[:, 8 * g + k:8 * g + k + 1], scalar2=None,
                        op0=ALU.is_equal)
                for k in range(8):
                    nc.tensor.matmul(
                        out=ghps[g][:, 0:96], lhsT=headsel[:],
                        rhs=ohg[:, 96 * k:96 * (k + 1)],
                        start=(k == 0), stop=(k == 7))
                for k in range(8):
                    nc.tensor.matmul(
                        out=ghps[g][:, 96:192],
                        lhsT=tailsel[:, 32 * k:32 * (k + 1)],
                        rhs=ohg[:, 96 * k:96 * (k + 1)],
                        start=(k == 0), stop=(k == 7))
                if g == 1:
                    nc.vector.tensor_copy(out=ghsb[1][:], in_=ghps[1][:])
            nc.vector.tensor_copy(out=ghsb[0][:], in_=ghps[0][:])

            # idx2 = 32*head2 + tail2 per graph-2 edge -> P2 one-hot
            idx2 = ps.tile([96, 1], F32, tag="psD", bufs=1)
            nc.tensor.matmul(out=idx2[:], lhsT=ghsb[1][:, 0:96],
                             rhs=wq[:, 0:1], start=True, stop=False)
            nc.tensor.matmul(out=idx2[:], lhsT=ghsb[1][:, 96:192],
                             rhs=wq[:, 1:2], start=False, stop=True)
            p2 = sb.tile([96, 1024], F16, tag="p2")
            for h in range(2):
                nc.vector.tensor_scalar(out=p2[:, 512 * h:512 * (h + 1)],
                                        in0=i1k16[:, 512 * h:512 * (h + 1)],
                                        scalar1=idx2[:, 0:1], scalar2=None,
                                        op0=ALU.is_equal)

            # ---- aux psum: [c | c' | MpT], DMA'd f32 directly ----
            aux = ps.tile([96, 34], F32, tag="psE", bufs=1)
            nc.tensor.matmul(out=aux[0:32, 2:34], lhsT=u2, rhs=u1,
                             start=True, stop=True)
            nc.tensor.matmul(out=aux[:, 0:2], lhsT=ghsb[0][:, 0:96],
                             rhs=wq[:, 2:4], start=True, stop=False)
            nc.tensor.matmul(out=aux[:, 0:2], lhsT=ghsb[0][:, 96:192],
                             rhs=wq[:, 4:6], start=False, stop=True)
            auxsb = sb.tile([96, 34], F16, tag="auxsb")
            nc.vector.tensor_copy(out=auxsb[:], in_=aux[:])
            nc.sync.dma_start(out=out_aux[:, :], in_=auxsb[:])

            # ================= C-chain (needs lamA + fu only) =============
            lp16 = []
            for i, l_ in enumerate((l1_16, l2_16)):
                lp_ = ps.tile([128, 128], F32, tag=f"psL{i}", bufs=1)
                nc.tensor.matmul(out=lp_[:], lhsT=id16[:], rhs=l_,
                                 start=True, stop=False)
                nc.tensor.matmul(out=lp_[:], lhsT=l_, rhs=id16[:],
                                 start=False, stop=True)
                l16_ = sb.tile([128, 128], F16, tag=f"lp16_{i}")
                nc.scalar.activation(out=l16_[:], in_=lp_[:], func=AF.Relu)
                lp16.append(l16_)
            # B = [lam1p @ F2 | lam2p @ F2]  (128, 64)
            bps = ps.tile([128, 64], F32, tag="psA", bufs=1)
            nc.tensor.matmul(out=bps[:, 0:32], lhsT=lp16[0][:], rhs=f2,
                             start=True, stop=True)
            nc.tensor.matmul(out=bps[:, 32:64], lhsT=lp16[1][:], rhs=f2,
                             start=True, stop=True)
            b16 = sb.tile([128, 64], F16, tag="b16")
            nc.scalar.copy(out=b16[:], in_=bps[:])
            # D_i = B_i^T F1 = C_i^T  (32a, 32c)
            dps = ps.tile([32, 64], F32, tag="psL0", bufs=1)
            nc.tensor.matmul(out=dps[:, 0:32], lhsT=b16[:, 0:32], rhs=f1,
                             start=True, stop=True)
            nc.tensor.matmul(out=dps[:, 32:64], lhsT=b16[:, 32:64], rhs=f1,
                             start=True, stop=True)
            d16 = sb.tile([32, 64], F16, tag="d16")
            nc.scalar.copy(out=d16[:], in_=dps[:])

            # ---- R|S = C1 G2 + C2 H2 | C2 G2 + C1 H2  (32, 192) ----
            rsps = ps.tile([32, 192], F32, tag="psA", bufs=1)
            g2sb, h2sb = ghsb[1][:, 0:96], ghsb[1][:, 96:192]
            nc.tensor.matmul(out=rsps[:, 0:96], lhsT=d16[:, 0:32], rhs=g2sb,
                             start=True, stop=False)
            nc.tensor.matmul(out=rsps[:, 0:96], lhsT=d16[:, 32:64], rhs=h2sb,
                             start=False, stop=True)
            nc.tensor.matmul(out=rsps[:, 96:192], lhsT=d16[:, 32:64],
                             rhs=g2sb, start=True, stop=False)
            nc.tensor.matmul(out=rsps[:, 96:192], lhsT=d16[:, 0:32],
                             rhs=h2sb, start=False, stop=True)
            rs16 = sb.tile([32, 192], F16, tag="rs16")
            nc.scalar.copy(out=rs16[:], in_=rsps[:])

            # ---- Me = G1^T R + H1^T S  (96 e1, 96 e2); the reference's
            # row-major scale flatten makes THIS the lhsT for Z ----
            meps = ps.tile([96, 96], F32, tag="psL1", bufs=1)
            nc.tensor.matmul(out=meps[:], lhsT=ghsb[0][:, 0:96],
                             rhs=rs16[:, 0:96], start=True, stop=False)
            nc.tensor.matmul(out=meps[:], lhsT=ghsb[0][:, 96:192],
                             rhs=rs16[:, 96:192], start=False, stop=True)
            me16 = sb.tile([96, 96], F16, tag="me16")
            nc.vector.tensor_copy(out=me16[:], in_=meps[:])

            # ---- Z = MeT^T-contraction @ P2; copy out in 256-chunks on
            # DVE/Act and DMA each 512-half as soon as it is in SBUF ----
            zsb = sb.tile([96, 1024], F16, tag="zsb")
            zps = ps.tile([96, 1024], F32, tag="psZ", bufs=1)
            for h in range(2):
                nc.tensor.matmul(out=zps[:, 512 * h:512 * (h + 1)],
                                 lhsT=me16[:],
                                 rhs=p2[:, 512 * h:512 * (h + 1)],
                                 start=True, stop=True)
                nc.vector.tensor_copy(
                    out=zsb[:, 512 * h:512 * h + 256],
                    in_=zps[:, 512 * h:512 * h + 256])
                nc.scalar.copy(
                    out=zsb[:, 512 * h + 256:512 * (h + 1)],
                    in_=zps[:, 512 * h + 256:512 * (h + 1)])
                nc.sync.dma_start(out=out_z[:, 512 * h:512 * (h + 1)],
                                  in_=zsb[:, 512 * h:512 * (h + 1)])
    nc.compile()
    return nc


def make_in_maps(inputs: dict) -> list:
    inputs = {k: np.asarray(v, dtype=np.float32) for k, v in inputs.items()}
    in_maps = []
    for b in range(B):
        lamA = np.zeros((128, 272), np.float16)
        lamA[:, 0:128] = inputs["lambda1"]
        lamA[:, 128:256] = inputs["lambda2"]
        lamA[:, 256:264] = inputs["A_src"][b].reshape(128, 8)
        lamA[:, 264:272] = inputs["A_tgt"][b].reshape(128, 8)
        fu = np.zeros((128, 128), np.float16)
        fu[:, 0:32] = inputs["F_src"][b]
        fu[:, 32:64] = inputs["F_tgt"][b]
        fu[:, 64:96] = inputs["U_src"][b]
        fu[:, 96:128] = inputs["U_tgt"][b]
        in_maps.append({
            "lamA": np.ascontiguousarray(lamA),
            "fu": np.ascontiguousarray(fu),
        })
    return in_maps


_NC_CACHE = {}


def _assemble(res: dict) -> np.ndarray:
    """Place device-computed Z values at device-computed (c, c') indices.

    out[(a,c(e)), (b,c'(e))] = Z[e,(a,b)]; out[i,i] += vec(Mp)[i].
    Pure placement + cast; no arithmetic on input data.
    """
    z = res["out_z"].astype(np.float32).reshape(E, 32, 32)
    aux = res["out_aux"].astype(np.float32)
    c = np.rint(aux[:, 0]).astype(np.int64)
    cp = np.rint(aux[:, 1]).astype(np.int64)
    mpt = aux[0:32, 2:34]                                # MpT[c, a]
    outm = np.zeros((1024, 1024), np.float32)
    o4 = outm.reshape(32, 32, 32, 32)
    o4[:, c, :, cp] = z                                  # axes (e, a, b)
    outm[np.arange(1024), np.arange(1024)] += mpt.T.ravel()
    return outm


def kernel(trace: bool = False, **inputs) -> np.ndarray:
    if "nc" not in _NC_CACHE:
        _NC_CACHE["nc"] = build_program()
    nc = _NC_CACHE["nc"]
    in_maps = make_in_maps(inputs)
    res = run_bass_kernel_spmd(nc, in_maps, core_ids=list(range(NCORES)),
                               trace=trace)
    _NC_CACHE["last_results"] = res
    outs = [_assemble(res.results[b]) for b in range(B)]
    return np.stack(outs).astype(np.float32)


# revision 17
# speedup vs baseline: 1.0498x; 1.0498x over previous
"""Trainium2 Bass kernel for nn_Affinity (graph-matching affinity matrix).

Math per sample (validated against the reference):
  out[(a,c),(b,c')] = sum_{e2,e1} G2[a,e2] H2[b,e2] Me[e2,e1] G1[c,e1] H1[c,e1]
                      + diag(vec(Mp))
(The reference's row-major flatten of Me makes the coefficient for the
(e2,e1) pair Me.flat[96*e2+e1], i.e. transpose-indexed.)

Structural facts exploited:
  * G/H columns are one-hot, so the dense 1024x1024 output is a placement
    of Z[e1,(a,b)] at rows/cols given by graph-1 edge endpoints, plus
    diag(vec(Mp)).
  * Me = G1^T R + H1^T S with R|S = C1 G2 + C2 H2 | C2 G2 + C1 H2 and
    C_i = F1^T relu(l_i + l_i^T) F2 (32x32).  C1/C2 depend only on
    lambda/F, so all d=128 contractions run during the adjacency DMA
    window; the adjacency-dependent path only does 32-contractions.
  * P2[e2, 32a+b] = (32*head2(e2) + tail2(e2) == 32a+b): a single
    per-partition one-hot against an iota, fed by matmuls against a
    per-slot value vector accumulated straight from the rank one-hots.

Device (1 sample per NeuronCore, fully static instruction stream):
  1. Row-major edge ranks via masked prefix-scan; one-hot rank expansion;
     G/H recovered by accumulating matmuls against head/tail select
     constants GENERATED on device during the DMA window (no DMA).
  2. C-chain on PE/Act in parallel with the rank chain.
  3. Z = Me-contraction with P2, copied out in 256-col chunks on DVE/Act
     and DMA'd per 512-col half.  aux = [c | c' | MpT] goes out early.
"""

import numpy as np

import concourse.bacc as bacc
import concourse.bass as bass
import concourse.mybir as mybir
import concourse.tile as tile
from concourse.bass_utils import run_bass_kernel_spmd

F32 = mybir.dt.float32
F16 = mybir.dt.float16
I32 = mybir.dt.int32
ALU = mybir.AluOpType
AX = mybir.AxisListType
AF = mybir.ActivationFunctionType

B, N, D, E = 8, 32, 128, 96
NCORES = 8


def build_program(debug: bool = False):
    nc = bacc.Bacc("TRN2", target_bir_lowering=False, debug=debug,
                   num_devices=NCORES)
    # lamA: lambda1 | lambda2 | A_src(128x8) | A_tgt(128x8), all f16-exact
    lamA = nc.dram_tensor("lamA", [128, 272], F16, kind="ExternalInput")
    # fu: F1 | F2 | U1 | U2  (each 128x32)
    fu = nc.dram_tensor("fu", [128, 128], F16, kind="ExternalInput")
    out_z = nc.dram_tensor("out_z", [96, 1024], F16, kind="ExternalOutput")
    out_aux = nc.dram_tensor("out_aux", [96, 34], F16, kind="ExternalOutput")

    with tile.TileContext(nc) as tc:
        with tc.tile_pool(name="sb", bufs=1) as sb, \
             tc.tile_pool(name="ps", bufs=1, space="PSUM") as ps:
            # ---- input DMAs (SP queue; HWDGE is serial across queues) ----
            lam_sb = sb.tile([128, 272], F16, tag="lam_sb")
            nc.sync.dma_start(out=lam_sb[:], in_=lamA[:, :])
            fu_sb = sb.tile([128, 128], F16, tag="fu_sb")
            nc.sync.dma_start(out=fu_sb[:], in_=fu[:, :])
            l1_16, l2_16 = lam_sb[:, 0:128], lam_sb[:, 128:256]
            a16 = lam_sb[:, 256:272]            # A_src cols 0:8, A_tgt 8:16
            f1, f2 = fu_sb[:, 0:32], fu_sb[:, 32:64]
            u1, u2 = fu_sb[:, 64:96], fu_sb[:, 96:128]

            # ---- on-device constants (input-DMA window) ----
            # Pool first: the big iota for the P2 one-hot compare.
            i1k32 = sb.tile([96, 1024], I32, tag="i1k32")
            nc.gpsimd.iota(i1k32[:], pattern=[[1, 1024]], base=0,
                           channel_multiplier=0)
            it32 = sb.tile([128, 129], I32, tag="it32")
            nc.gpsimd.iota(it32[:], pattern=[[1, 129]], base=0,
                           channel_multiplier=0)
            pi32 = sb.tile([128, 1], I32, tag="pi32")
            nc.gpsimd.iota(pi32[:], pattern=[[1, 1]], base=0,
                           channel_multiplier=1)
            io16 = sb.tile([128, 129], F16, tag="io16")
            nc.vector.tensor_copy(out=io16[:], in_=it32[:])
            io32 = sb.tile([128, 128], F32, tag="io32")
            nc.vector.tensor_copy(out=io32[:], in_=it32[:, 0:128])
            pf32 = sb.tile([128, 1], F32, tag="pf32")
            nc.vector.tensor_copy(out=pf32[:], in_=pi32[:])
            pf16 = sb.tile([128, 1], F16, tag="pf16")
            nc.vector.tensor_copy(out=pf16[:], in_=pi32[:])
            id16 = sb.tile([128, 128], F16, tag="id16")
            nc.vector.tensor_scalar(out=id16[:], in0=io16[:, 0:128],
                                    scalar1=pf32[:, 0:1], scalar2=None,
                                    op0=ALU.is_equal)
            su32 = sb.tile([128, 128], F32, tag="su32")
            nc.vector.tensor_scalar(out=su32[:], in0=io32[:],
                                    scalar1=pf32[:, 0:1], scalar2=None,
                                    op0=ALU.is_gt)
            # head/tail select constants: head row = p//4,
            # tail col of flat slot (p,k) = 8*(p%4)+k
            q32 = sb.tile([128, 1], I32, tag="q32")
            nc.vector.tensor_scalar(out=q32[:], in0=pi32[:], scalar1=2,
                                    scalar2=None,
                                    op0=ALU.logical_shift_right)
            qf32 = sb.tile([128, 1], F32, tag="qf32")
            nc.vector.tensor_copy(out=qf32[:], in_=q32[:])
            headsel = sb.tile([128, 32], F16, tag="headsel")
            nc.vector.tensor_scalar(out=headsel[:], in0=io16[:, 0:32],
                                    scalar1=qf32[:, 0:1], scalar2=None,
                                    op0=ALU.is_equal)
            m32 = sb.tile([128, 1], I32, tag="m32")
            nc.vector.tensor_scalar(out=m32[:], in0=pi32[:], scalar1=3,
                                    scalar2=None, op0=ALU.bitwise_and)
            b832 = sb.tile([128, 1], I32, tag="b832")
            nc.vector.tensor_scalar(out=b832[:], in0=m32[:], scalar1=3,
                                    scalar2=None, op0=ALU.logical_shift_left)
            b8f = sb.tile([128, 1], F32, tag="b8f")
            nc.vector.tensor_copy(out=b8f[:], in_=b832[:])
            tailsel = sb.tile([128, 256], F16, tag="tailsel")
            for k in range(8):
                nc.vector.tensor_scalar(out=tailsel[:, 32 * k:32 * (k + 1)],
                                        in0=io16[:, 0:32],
                                        scalar1=b8f[:, 0:1],
                                        scalar2=float(k),
                                        op0=ALU.subtract, op1=ALU.is_equal)
            # per-slot flat value 32*head + tail(k) for the idx2 matmuls
            vb = sb.tile([128, 1], F32, tag="vb")
            nc.vector.scalar_tensor_tensor(out=vb[:], in0=qf32[:],
                                           scalar=32.0, in1=b8f[:],
                                           op0=ALU.mult, op1=ALU.add)
            v16 = sb.tile([128, 8], F16, tag="v16")
            nc.vector.tensor_scalar(out=v16[:], in0=io16[:, 0:8],
                                    scalar1=vb[:, 0:1], scalar2=None,
                                    op0=ALU.add)
            # eps weights: wq = [p | 0 | 0 | p] on first 32 partitions
            wq = sb.tile([32, 4], F16, tag="wq")
            nc.vector.memset(wq[:], 0.0)
            nc.vector.tensor_copy(out=wq[:, 0:1], in_=pf16[0:32, :])
            nc.vector.tensor_copy(out=wq[:, 3:4], in_=pf16[0:32, :])

            # ================= C-chain (needs lamA + fu only) =============
            # lamp = [l1+l1^T | l2+l2^T] in one psum; one fused relu copy
            lamp = ps.tile([128, 256], F32, tag="psL", bufs=1)
            for i, l_ in enumerate((l1_16, l2_16)):
                nc.tensor.matmul(out=lamp[:, 128 * i:128 * (i + 1)],
                                 lhsT=id16[:], rhs=l_,
                                 start=True, stop=False)
                nc.tensor.matmul(out=lamp[:, 128 * i:128 * (i + 1)],
                                 lhsT=l_, rhs=id16[:],
                                 start=False, stop=True)
            lp16 = sb.tile([128, 256], F16, tag="lp16")
            nc.scalar.activation(out=lp16[:], in_=lamp[:], func=AF.Relu)
            # B = [lam1p @ F2 | lam2p @ F2]  (128, 64)
            bps = ps.tile([128, 64], F32, tag="psA", bufs=1)
            nc.tensor.matmul(out=bps[:, 0:32], lhsT=lp16[:, 0:128], rhs=f2,
                             start=True, stop=True)
            nc.tensor.matmul(out=bps[:, 32:64], lhsT=lp16[:, 128:256],
                             rhs=f2, start=True, stop=True)
            b16 = sb.tile([128, 64], F16, tag="b16")
            nc.scalar.copy(out=b16[:], in_=bps[:])
            # D_i = B_i^T F1 = C_i^T  (32a, 32c)
            dps = ps.tile([32, 64], F32, tag="psL", bufs=1)
            nc.tensor.matmul(out=dps[:, 0:32], lhsT=b16[:, 0:32], rhs=f1,
                             start=True, stop=True)
            nc.tensor.matmul(out=dps[:, 32:64], lhsT=b16[:, 32:64], rhs=f1,
                             start=True, stop=True)
            d16 = sb.tile([32, 64], F16, tag="d16")
            nc.scalar.copy(out=d16[:], in_=dps[:])

            # ================= rank chain (needs lamA) =================
            maskb = sb.tile([128, 16], F32, tag="maskb")
            nc.vector.tensor_copy(out=maskb[:], in_=a16)
            s2 = sb.tile([128, 2], F32, tag="s2")
            m3 = maskb[:].rearrange("p (g k) -> p g k", k=8)
            nc.vector.tensor_reduce(out=s2[:], in_=m3, axis=AX.X, op=ALU.add)
            pb = ps.tile([128, 2], F32, tag="psI", bufs=1)
            nc.tensor.matmul(out=pb[:], lhsT=su32[:], rhs=s2[:],
                             start=True, stop=True)
            r0 = sb.tile([128, 16], F32, tag="r0")
            r1h = sb.tile([128, 16], F32, tag="r1h")
            for g in (1, 0):
                nc.vector.tensor_tensor_scan(
                    out=r0[:, 8 * g:8 * (g + 1)],
                    data0=maskb[:, 8 * g:8 * (g + 1)],
                    data1=maskb[:, 8 * g:8 * (g + 1)],
                    initial=pb[:, g:g + 1],
                    op0=ALU.add, op1=ALU.bypass)
                nc.vector.tensor_tensor(out=r1h[:, 8 * g:8 * (g + 1)],
                                        in0=r0[:, 8 * g:8 * (g + 1)],
                                        in1=maskb[:, 8 * g:8 * (g + 1)],
                                        op=ALU.mult)

            # ---- graph2: one-hots + G2/H2/idx2 accumulating matmuls ----
            oh2t = sb.tile([128, 768], F16, tag="oh2")
            gh2ps = ps.tile([32, 192], F32, tag="psG2", bufs=1)
            idx2 = ps.tile([96, 1], F32, tag="psI", bufs=1)
            for k in range(8):
                nc.vector.tensor_scalar(
                    out=oh2t[:, 96 * k:96 * (k + 1)],
                    in0=io16[:, 1:97],
                    scalar1=r1h[:, 8 + k:9 + k], scalar2=None,
                    op0=ALU.is_equal)
            for k in range(8):
                nc.tensor.matmul(out=gh2ps[:, 0:96], lhsT=headsel[:],
                                 rhs=oh2t[:, 96 * k:96 * (k + 1)],
                                 start=(k == 0), stop=(k == 7))
            for k in range(8):
                nc.tensor.matmul(out=gh2ps[:, 96:192],
                                 lhsT=tailsel[:, 32 * k:32 * (k + 1)],
                                 rhs=oh2t[:, 96 * k:96 * (k + 1)],
                                 start=(k == 0), stop=(k == 7))
            for k in range(8):
                nc.tensor.matmul(out=idx2[:],
                                 lhsT=oh2t[:, 96 * k:96 * (k + 1)],
                                 rhs=v16[:, k:k + 1],
                                 start=(k == 0), stop=(k == 7))
            gh2t = sb.tile([32, 192], F16, tag="gh2sb")
            nc.scalar.copy(out=gh2t[:], in_=gh2ps[:])

            # ---- graph1: one-hots + G1/H1 matmuls ----
            oh1t = sb.tile([128, 768], F16, tag="oh1")
            gh1ps = ps.tile([32, 192], F32, tag="psG1", bufs=1)
            for k in range(8):
                nc.vector.tensor_scalar(
                    out=oh1t[:, 96 * k:96 * (k + 1)],
                    in0=io16[:, 1:97],
                    scalar1=r1h[:, k:k + 1], scalar2=None,
                    op0=ALU.is_equal)
            for k in range(8):
                nc.tensor.matmul(out=gh1ps[:, 0:96], lhsT=headsel[:],
                                 rhs=oh1t[:, 96 * k:96 * (k + 1)],
                                 start=(k == 0), stop=(k == 7))
            for k in range(8):
                nc.tensor.matmul(out=gh1ps[:, 96:192],
                                 lhsT=tailsel[:, 32 * k:32 * (k + 1)],
                                 rhs=oh1t[:, 96 * k:96 * (k + 1)],
                                 start=(k == 0), stop=(k == 7))
            gh1t = sb.tile([32, 192], F16, tag="gh1sb")
            nc.vector.tensor_copy(out=gh1t[:], in_=gh1ps[:])

            # P2 one-hot from idx2 (i32 iota compared against f32 scalar)
            p2 = sb.tile([96, 1024], F16, tag="p2")
            for h in range(2):
                nc.vector.tensor_scalar(out=p2[:, 512 * h:512 * (h + 1)],
                                        in0=i1k32[:, 512 * h:512 * (h + 1)],
                                        scalar1=idx2[:, 0:1], scalar2=None,
                                        op0=ALU.is_equal)

            # ---- R|S = C1 G2 + C2 H2 | C2 G2 + C1 H2  (32, 192) ----
            rsps = ps.tile([32, 192], F32, tag="psA", bufs=1)
            g2sb, h2sb = gh2t[:, 0:96], gh2t[:, 96:192]
            nc.tensor.matmul(out=rsps[:, 0:96], lhsT=d16[:, 0:32], rhs=g2sb,
                             start=True, stop=False)
            nc.tensor.matmul(out=rsps[:, 0:96], lhsT=d16[:, 32:64], rhs=h2sb,
                             start=False, stop=True)
            nc.tensor.matmul(out=rsps[:, 96:192], lhsT=d16[:, 32:64],
                             rhs=g2sb, start=True, stop=False)
            nc.tensor.matmul(out=rsps[:, 96:192], lhsT=d16[:, 0:32],
                             rhs=h2sb, start=False, stop=True)
            rs16 = sb.tile([32, 192], F16, tag="rs16")
            nc.scalar.copy(out=rs16[:], in_=rsps[:])

            # ---- Me = G1^T R + H1^T S  (96 e1, 96 e2) ----
            meps = ps.tile([96, 96], F32, tag="psL", bufs=1)
            nc.tensor.matmul(out=meps[:], lhsT=gh1t[:, 0:96],
                             rhs=rs16[:, 0:96], start=True, stop=False)
            nc.tensor.matmul(out=meps[:], lhsT=gh1t[:, 96:192],
                             rhs=rs16[:, 96:192], start=False, stop=True)
            me16 = sb.tile([96, 96], F16, tag="me16")
            nc.scalar.copy(out=me16[:], in_=meps[:])

            # ---- Z = Me-contraction @ P2; 256-chunk copies, 512 DMAs ----
            zsb = sb.tile([96, 1024], F16, tag="zsb")
            zp0 = ps.tile([96, 512], F32, tag="psG2", bufs=1)
            zp1 = ps.tile([96, 512], F32, tag="psH2", bufs=1)
            for h, zp in enumerate((zp0, zp1)):
                nc.tensor.matmul(out=zp[:], lhsT=me16[:],
                                 rhs=p2[:, 512 * h:512 * (h + 1)],
                                 start=True, stop=True)
                nc.vector.tensor_copy(
                    out=zsb[:, 512 * h:512 * h + 256],
                    in_=zp[:, 0:256])
                nc.scalar.copy(
                    out=zsb[:, 512 * h + 256:512 * (h + 1)],
                    in_=zp[:, 256:512])
                nc.sync.dma_start(out=out_z[:, 512 * h:512 * (h + 1)],
                                  in_=zsb[:, 512 * h:512 * (h + 1)])

            # ---- aux = [c | c' | MpT]; off-critical, early DMA ----
            aux = ps.tile([96, 34], F32, tag="psE", bufs=1)
            nc.tensor.matmul(out=aux[:, 0:2], lhsT=gh1t[:, 0:96],
                             rhs=wq[:, 0:2], start=True, stop=False)
            nc.tensor.matmul(out=aux[:, 0:2], lhsT=gh1t[:, 96:192],
                             rhs=wq[:, 2:4], start=False, stop=True)
            nc.tensor.matmul(out=aux[0:32, 2:34], lhsT=u2, rhs=u1,
                             start=True, stop=True)
            auxsb = sb.tile([96, 34], F16, tag="auxsb")
            nc.vector.tensor_copy(out=auxsb[:], in_=aux[:])
            nc.sync.dma_start(out=out_aux[:, :], in_=auxsb[:])
    nc.compile()
    return nc


def make_in_maps(inputs: dict) -> list:
    inputs = {k: np.asarray(v, dtype=np.float32) for k, v in inputs.items()}
    in_maps = []
    for b in range(B):
        lamA = np.zeros((128, 272), np.float16)
        lamA[:, 0:128] = inputs["lambda1"]
        lamA[:, 128:256] = inputs["lambda2"]
        lamA[:, 256:264] = inputs["A_src"][b].reshape(128, 8)
        lamA[:, 264:272] = inputs["A_tgt"][b].reshape(128, 8)
        fu = np.zeros((128, 128), np.float16)
        fu[:, 0:32] = inputs["F_src"][b]
        fu[:, 32:64] = inputs["F_tgt"][b]
        fu[:, 64:96] = inputs["U_src"][b]
        fu[:, 96:128] = inputs["U_tgt"][b]
        in_maps.append({
            "lamA": np.ascontiguousarray(lamA),
            "fu": np.ascontiguousarray(fu),
        })
    return in_maps


_NC_CACHE = {}


def _assemble(res: dict) -> np.ndarray:
    """Place device-computed Z values at device-computed (c, c') indices.

    out[(a,c(e)), (b,c'(e))] = Z[e,(a,b)]; out[i,i] += vec(Mp)[i].
    Pure placement + cast; no arithmetic on input data.
    """
    z = res["out_z"].astype(np.float32).reshape(E, 32, 32)
    aux = res["out_aux"].astype(np.float32)
    c = np.rint(aux[:, 0]).astype(np.int64)
    cp = np.rint(aux[:, 1]).astype(np.int64)
    mpt = aux[0:32, 2:34]                                # MpT[c, a]
    outm = np.zeros((1024, 1024), np.float32)
    o4 = outm.reshape(32, 32, 32, 32)
    o4[:, c, :, cp] = z                                  # axes (e, a, b)
    outm[np.arange(1024), np.arange(1024)] += mpt.T.ravel()
    return outm


def kernel(trace: bool = False, **inputs) -> np.ndarray:
    if "nc" not in _NC_CACHE:
        _NC_CACHE["nc"] = build_program()
    nc = _NC_CACHE["nc"]
    in_maps = make_in_maps(inputs)
    res = run_bass_kernel_spmd(nc, in_maps, core_ids=list(range(NCORES)),
                               trace=trace)
    _NC_CACHE["last_results"] = res
    outs = [_assemble(res.results[b]) for b in range(B)]
    return np.stack(outs).astype(np.float32)


# revision 18
# speedup vs baseline: 1.1054x; 1.0530x over previous
"""Trainium2 Bass kernel for nn_Affinity (graph-matching affinity matrix).

Math per sample (validated against the reference):
  out[(a,c),(b,c')] = sum_{e2,e1} G2[a,e2] H2[b,e2] Me[e2,e1] G1[c,e1] H1[c,e1]
                      + diag(vec(Mp))
(The reference's row-major flatten of Me pairs row-position t of the
e1-enumeration with row-position t of the e2-enumeration; the device
contraction below reproduces it exactly and is validated end-to-end.)

Structural facts exploited:
  * G/H columns are one-hot, so the dense 1024x1024 output is a placement
    of Z[e1,(a,b)] at rows/cols given by graph-1 edge endpoints, plus
    diag(vec(Mp)).
  * Me = G1^T R + H1^T S with R|S = C1 G2 + C2 H2 | C2 G2 + C1 H2 and
    C_i = F1^T relu(l_i + l_i^T) F2 (32x32).  C1/C2 depend only on
    lambda/F, so all d=128 contractions run during the input-DMA window
    and concurrently with the rank chain; the adjacency-dependent path
    only does 32-contractions.
  * P2[e2, 32a+b] = (32*head2(e2) + tail2(e2) == 32a+b): a single
    per-partition one-hot against an iota, fed by matmuls against a
    per-slot value vector accumulated straight from the rank one-hots.

Device (1 sample per NeuronCore, fully static instruction stream):
  1. ONE packed input DMA (lambda | A | F | U) so the tile scheduler sees
     uniform readiness; select/iota constants generated on device during
     the DMA window (DVE + Pool split).
  2. C-chain on PE/Act in parallel with the rank chain on DVE/PE.
  3. Z = Me-contraction with P2 in 4x256 chunks, copies alternating
     DVE/Act, DMA per 512-half.  aux = [c | c' | MpT] goes out early.
"""

import numpy as np

import concourse.bacc as bacc
import concourse.bass as bass
import concourse.mybir as mybir
import concourse.tile as tile
from concourse.bass_utils import run_bass_kernel_spmd

F32 = mybir.dt.float32
F16 = mybir.dt.float16
I32 = mybir.dt.int32
ALU = mybir.AluOpType
AX = mybir.AxisListType
AF = mybir.ActivationFunctionType

B, N, D, E = 8, 32, 128, 96
NCORES = 8


def build_program(debug: bool = False):
    nc = bacc.Bacc("TRN2", target_bir_lowering=False, debug=debug,
                   num_devices=NCORES)
    # inp: lambda1 | lambda2 | A(16) | F1 | F2 | U1 | U2   (128, 400) f16
    inp = nc.dram_tensor("inp", [128, 400], F16, kind="ExternalInput")
    out_z = nc.dram_tensor("out_z", [96, 1024], F16, kind="ExternalOutput")
    out_aux = nc.dram_tensor("out_aux", [96, 34], F16, kind="ExternalOutput")

    with tile.TileContext(nc) as tc:
        with tc.tile_pool(name="sb", bufs=1) as sb, \
             tc.tile_pool(name="ps", bufs=1, space="PSUM") as ps:
            in_sb = sb.tile([128, 400], F16, tag="in_sb")
            nc.sync.dma_start(out=in_sb[:], in_=inp[:, :])
            l1_16, l2_16 = in_sb[:, 0:128], in_sb[:, 128:256]
            a16 = in_sb[:, 256:272]             # A_src cols 0:8, A_tgt 8:16
            f1, f2 = in_sb[:, 272:304], in_sb[:, 304:336]
            u1, u2 = in_sb[:, 336:368], in_sb[:, 368:400]

            # ---- on-device constants (input-DMA window) ----
            it32 = sb.tile([128, 129], I32, tag="it32")
            nc.gpsimd.iota(it32[:], pattern=[[1, 129]], base=0,
                           channel_multiplier=0)
            pi32 = sb.tile([128, 1], I32, tag="pi32")
            nc.gpsimd.iota(pi32[:], pattern=[[1, 1]], base=0,
                           channel_multiplier=1)
            i1k32 = sb.tile([96, 1024], I32, tag="i1k32")
            nc.gpsimd.iota(i1k32[:], pattern=[[1, 1024]], base=0,
                           channel_multiplier=0)
            # DVE-side small constants
            io16 = sb.tile([128, 129], F16, tag="io16")
            nc.vector.tensor_copy(out=io16[:], in_=it32[:])
            io32 = sb.tile([128, 128], F32, tag="io32")
            nc.vector.tensor_copy(out=io32[:], in_=it32[:, 0:128])
            pf32 = sb.tile([128, 1], F32, tag="pf32")
            nc.vector.tensor_copy(out=pf32[:], in_=pi32[:])
            pf16 = sb.tile([128, 1], F16, tag="pf16")
            nc.vector.tensor_copy(out=pf16[:], in_=pi32[:])
            q32 = sb.tile([128, 1], I32, tag="q32")
            nc.vector.tensor_scalar(out=q32[:], in0=pi32[:], scalar1=2,
                                    scalar2=None,
                                    op0=ALU.logical_shift_right)
            qf32 = sb.tile([128, 1], F32, tag="qf32")
            nc.vector.tensor_copy(out=qf32[:], in_=q32[:])
            headsel = sb.tile([128, 32], F16, tag="headsel")
            nc.vector.tensor_scalar(out=headsel[:], in0=io16[:, 0:32],
                                    scalar1=qf32[:, 0:1], scalar2=None,
                                    op0=ALU.is_equal)
            m32 = sb.tile([128, 1], I32, tag="m32")
            nc.vector.tensor_scalar(out=m32[:], in0=pi32[:], scalar1=3,
                                    scalar2=None, op0=ALU.bitwise_and)
            b832 = sb.tile([128, 1], I32, tag="b832")
            nc.vector.tensor_scalar(out=b832[:], in0=m32[:], scalar1=3,
                                    scalar2=None, op0=ALU.logical_shift_left)
            b8f = sb.tile([128, 1], F32, tag="b8f")
            nc.vector.tensor_copy(out=b8f[:], in_=b832[:])
            vb = sb.tile([128, 1], F32, tag="vb")
            nc.vector.scalar_tensor_tensor(out=vb[:], in0=qf32[:],
                                           scalar=32.0, in1=b8f[:],
                                           op0=ALU.mult, op1=ALU.add)
            v16 = sb.tile([128, 8], F16, tag="v16")
            nc.vector.tensor_scalar(out=v16[:], in0=io16[:, 0:8],
                                    scalar1=vb[:, 0:1], scalar2=None,
                                    op0=ALU.add)
            wq = sb.tile([32, 4], F16, tag="wq")
            nc.vector.memset(wq[:], 0.0)
            nc.vector.tensor_copy(out=wq[:, 0:1], in_=pf16[0:32, :])
            nc.vector.tensor_copy(out=wq[:, 3:4], in_=pf16[0:32, :])
            # Pool-side bigger constants (after the iotas)
            id16 = sb.tile([128, 128], F16, tag="id16")
            nc.gpsimd.tensor_scalar(out=id16[:], in0=io16[:, 0:128],
                                    scalar1=pf32[:, 0:1], scalar2=None,
                                    op0=ALU.is_equal)
            su32 = sb.tile([128, 128], F32, tag="su32")
            nc.gpsimd.tensor_scalar(out=su32[:], in0=io32[:],
                                    scalar1=pf32[:, 0:1], scalar2=None,
                                    op0=ALU.is_gt)
            tailsel = sb.tile([128, 256], F16, tag="tailsel")
            for k in range(8):
                nc.gpsimd.tensor_scalar(out=tailsel[:, 32 * k:32 * (k + 1)],
                                        in0=io16[:, 0:32],
                                        scalar1=b8f[:, 0:1],
                                        scalar2=float(k),
                                        op0=ALU.subtract, op1=ALU.is_equal)

            # ================= C-chain (PE + Act) =================
            lamp = ps.tile([128, 256], F32, tag="psL", bufs=1)
            for i, l_ in enumerate((l1_16, l2_16)):
                nc.tensor.matmul(out=lamp[:, 128 * i:128 * (i + 1)],
                                 lhsT=id16[:], rhs=l_,
                                 start=True, stop=False)
                nc.tensor.matmul(out=lamp[:, 128 * i:128 * (i + 1)],
                                 lhsT=l_, rhs=id16[:],
                                 start=False, stop=True)
            lp16 = sb.tile([128, 256], F16, tag="lp16")
            nc.scalar.activation(out=lp16[:], in_=lamp[:], func=AF.Relu)
            bps = ps.tile([128, 64], F32, tag="psA", bufs=1)
            nc.tensor.matmul(out=bps[:, 0:32], lhsT=lp16[:, 0:128], rhs=f2,
                             start=True, stop=True)
            nc.tensor.matmul(out=bps[:, 32:64], lhsT=lp16[:, 128:256],
                             rhs=f2, start=True, stop=True)
            b16 = sb.tile([128, 64], F16, tag="b16")
            nc.scalar.copy(out=b16[:], in_=bps[:])
            dps = ps.tile([32, 64], F32, tag="psL", bufs=1)
            nc.tensor.matmul(out=dps[:, 0:32], lhsT=b16[:, 0:32], rhs=f1,
                             start=True, stop=True)
            nc.tensor.matmul(out=dps[:, 32:64], lhsT=b16[:, 32:64], rhs=f1,
                             start=True, stop=True)
            d16 = sb.tile([32, 64], F16, tag="d16")
            nc.scalar.copy(out=d16[:], in_=dps[:])

            # ================= rank chain =================
            maskb = sb.tile([128, 16], F32, tag="maskb")
            nc.vector.tensor_copy(out=maskb[:], in_=a16)
            s2 = sb.tile([128, 2], F32, tag="s2")
            m3 = maskb[:].rearrange("p (g k) -> p g k", k=8)
            nc.vector.tensor_reduce(out=s2[:], in_=m3, axis=AX.X, op=ALU.add)
            pb = ps.tile([128, 2], F32, tag="psI", bufs=1)
            nc.tensor.matmul(out=pb[:], lhsT=su32[:], rhs=s2[:],
                             start=True, stop=True)
            r0 = sb.tile([128, 16], F32, tag="r0")
            r1h = sb.tile([128, 16], F32, tag="r1h")
            for g in (1, 0):
                nc.vector.tensor_tensor_scan(
                    out=r0[:, 8 * g:8 * (g + 1)],
                    data0=maskb[:, 8 * g:8 * (g + 1)],
                    data1=maskb[:, 8 * g:8 * (g + 1)],
                    initial=pb[:, g:g + 1],
                    op0=ALU.add, op1=ALU.bypass)
                nc.vector.tensor_tensor(out=r1h[:, 8 * g:8 * (g + 1)],
                                        in0=r0[:, 8 * g:8 * (g + 1)],
                                        in1=maskb[:, 8 * g:8 * (g + 1)],
                                        op=ALU.mult)

            # ---- graph2: one-hots + G2/H2/idx2 accumulating matmuls ----
            oh2t = sb.tile([128, 768], F16, tag="oh2")
            gh2ps = ps.tile([32, 192], F32, tag="psG2", bufs=1)
            idx2 = ps.tile([96, 1], F32, tag="psI", bufs=1)
            for k in range(8):
                nc.vector.tensor_scalar(
                    out=oh2t[:, 96 * k:96 * (k + 1)],
                    in0=io16[:, 1:97],
                    scalar1=r1h[:, 8 + k:9 + k], scalar2=None,
                    op0=ALU.is_equal)
            for k in range(8):
                nc.tensor.matmul(out=gh2ps[:, 0:96], lhsT=headsel[:],
                                 rhs=oh2t[:, 96 * k:96 * (k + 1)],
                                 start=(k == 0), stop=(k == 7))
            for k in range(8):
                nc.tensor.matmul(out=gh2ps[:, 96:192],
                                 lhsT=tailsel[:, 32 * k:32 * (k + 1)],
                                 rhs=oh2t[:, 96 * k:96 * (k + 1)],
                                 start=(k == 0), stop=(k == 7))
            for k in range(8):
                nc.tensor.matmul(out=idx2[:],
                                 lhsT=oh2t[:, 96 * k:96 * (k + 1)],
                                 rhs=v16[:, k:k + 1],
                                 start=(k == 0), stop=(k == 7))
            gh2t = sb.tile([32, 192], F16, tag="gh2sb")
            nc.vector.tensor_copy(out=gh2t[:], in_=gh2ps[:])

            # ---- graph1: one-hots + G1/H1 matmuls ----
            oh1t = sb.tile([128, 768], F16, tag="oh1")
            gh1ps = ps.tile([32, 192], F32, tag="psG1", bufs=1)
            for k in range(8):
                nc.vector.tensor_scalar(
                    out=oh1t[:, 96 * k:96 * (k + 1)],
                    in0=io16[:, 1:97],
                    scalar1=r1h[:, k:k + 1], scalar2=None,
                    op0=ALU.is_equal)
            for k in range(8):
                nc.tensor.matmul(out=gh1ps[:, 0:96], lhsT=headsel[:],
                                 rhs=oh1t[:, 96 * k:96 * (k + 1)],
                                 start=(k == 0), stop=(k == 7))
            for k in range(8):
                nc.tensor.matmul(out=gh1ps[:, 96:192],
                                 lhsT=tailsel[:, 32 * k:32 * (k + 1)],
                                 rhs=oh1t[:, 96 * k:96 * (k + 1)],
                                 start=(k == 0), stop=(k == 7))

            # P2 one-hot from idx2 (i32 iota compared against f32 scalar)
            p2 = sb.tile([96, 1024], F16, tag="p2")
            for h in range(2):
                nc.vector.tensor_scalar(out=p2[:, 512 * h:512 * (h + 1)],
                                        in0=i1k32[:, 512 * h:512 * (h + 1)],
                                        scalar1=idx2[:, 0:1], scalar2=None,
                                        op0=ALU.is_equal)
            gh1t = sb.tile([32, 192], F16, tag="gh1sb")
            nc.vector.tensor_copy(out=gh1t[:], in_=gh1ps[:])

            # ---- R|S = C1 G2 + C2 H2 | C2 G2 + C1 H2  (32, 192) ----
            rsps = ps.tile([32, 192], F32, tag="psA", bufs=1)
            g2sb, h2sb = gh2t[:, 0:96], gh2t[:, 96:192]
            nc.tensor.matmul(out=rsps[:, 0:96], lhsT=d16[:, 0:32], rhs=g2sb,
                             start=True, stop=False)
            nc.tensor.matmul(out=rsps[:, 0:96], lhsT=d16[:, 32:64], rhs=h2sb,
                             start=False, stop=True)
            nc.tensor.matmul(out=rsps[:, 96:192], lhsT=d16[:, 32:64],
                             rhs=g2sb, start=True, stop=False)
            nc.tensor.matmul(out=rsps[:, 96:192], lhsT=d16[:, 0:32],
                             rhs=h2sb, start=False, stop=True)
            rs16 = sb.tile([32, 192], F16, tag="rs16")
            nc.scalar.copy(out=rs16[:], in_=rsps[:])

            # ---- Me = G1^T R + H1^T S  (96, 96) ----
            meps = ps.tile([96, 96], F32, tag="psL", bufs=1)
            nc.tensor.matmul(out=meps[:], lhsT=gh1t[:, 0:96],
                             rhs=rs16[:, 0:96], start=True, stop=False)
            nc.tensor.matmul(out=meps[:], lhsT=gh1t[:, 96:192],
                             rhs=rs16[:, 96:192], start=False, stop=True)
            me16 = sb.tile([96, 96], F16, tag="me16")
            nc.vector.tensor_copy(out=me16[:], in_=meps[:])

            # ---- aux = [c | c' | MpT]; off-critical, early DMA ----
            aux = ps.tile([96, 34], F32, tag="psE", bufs=1)
            nc.tensor.matmul(out=aux[:, 0:2], lhsT=gh1t[:, 0:96],
                             rhs=wq[:, 0:2], start=True, stop=False)
            nc.tensor.matmul(out=aux[:, 0:2], lhsT=gh1t[:, 96:192],
                             rhs=wq[:, 2:4], start=False, stop=True)
            nc.tensor.matmul(out=aux[0:32, 2:34], lhsT=u2, rhs=u1,
                             start=True, stop=True)
            auxsb = sb.tile([96, 34], F16, tag="auxsb")
            nc.vector.tensor_copy(out=auxsb[:], in_=aux[:])
            nc.sync.dma_start(out=out_aux[:, :], in_=auxsb[:])

            # ---- Z in 4x256 chunks; copies DVE/Act; DMA per 512-half ----
            zsb = sb.tile([96, 1024], F16, tag="zsb")
            zp0 = ps.tile([96, 512], F32, tag="psG2", bufs=1)
            zp1 = ps.tile([96, 512], F32, tag="psH2", bufs=1)
            zp = {0: zp0, 1: zp1}
            for q in range(4):
                h, c = q // 2, q % 2
                nc.tensor.matmul(out=zp[h][:, 256 * c:256 * (c + 1)],
                                 lhsT=me16[:],
                                 rhs=p2[:, 256 * q:256 * (q + 1)],
                                 start=True, stop=True)
                eng = nc.vector.tensor_copy if q % 2 == 0 else (
                    lambda out, in_: nc.scalar.copy(out=out, in_=in_))
                eng(out=zsb[:, 256 * q:256 * (q + 1)],
                    in_=zp[h][:, 256 * c:256 * (c + 1)])
                if q % 2 == 1:
                    nc.sync.dma_start(
                        out=out_z[:, 512 * h:512 * (h + 1)],
                        in_=zsb[:, 512 * h:512 * (h + 1)])
    nc.compile()
    return nc


def make_in_maps(inputs: dict) -> list:
    inputs = {k: np.asarray(v, dtype=np.float32) for k, v in inputs.items()}
    in_maps = []
    for b in range(B):
        pk = np.zeros((128, 400), np.float16)
        pk[:, 0:128] = inputs["lambda1"]
        pk[:, 128:256] = inputs["lambda2"]
        pk[:, 256:264] = inputs["A_src"][b].reshape(128, 8)
        pk[:, 264:272] = inputs["A_tgt"][b].reshape(128, 8)
        pk[:, 272:304] = inputs["F_src"][b]
        pk[:, 304:336] = inputs["F_tgt"][b]
        pk[:, 336:368] = inputs["U_src"][b]
        pk[:, 368:400] = inputs["U_tgt"][b]
        in_maps.append({"inp": np.ascontiguousarray(pk)})
    return in_maps


_NC_CACHE = {}


def _assemble(res: dict) -> np.ndarray:
    """Place device-computed Z values at device-computed (c, c') indices.

    out[(a,c(e)), (b,c'(e))] = Z[e,(a,b)]; out[i,i] += vec(Mp)[i].
    Pure placement + cast; no arithmetic on input data.
    """
    z = res["out_z"].astype(np.float32).reshape(E, 32, 32)
    aux = res["out_aux"].astype(np.float32)
    c = np.rint(aux[:, 0]).astype(np.int64)
    cp = np.rint(aux[:, 1]).astype(np.int64)
    mpt = aux[0:32, 2:34]                                # MpT[c, a]
    outm = np.zeros((1024, 1024), np.float32)
    o4 = outm.reshape(32, 32, 32, 32)
    o4[:, c, :, cp] = z                                  # axes (e, a, b)
    outm[np.arange(1024), np.arange(1024)] += mpt.T.ravel()
    return outm


def kernel(trace: bool = False, **inputs) -> np.ndarray:
    if "nc" not in _NC_CACHE:
        _NC_CACHE["nc"] = build_program()
    nc = _NC_CACHE["nc"]
    in_maps = make_in_maps(inputs)
    res = run_bass_kernel_spmd(nc, in_maps, core_ids=list(range(NCORES)),
                               trace=trace)
    _NC_CACHE["last_results"] = res
    outs = [_assemble(res.results[b]) for b in range(B)]
    return np.stack(outs).astype(np.float32)


# revision 19
# speedup vs baseline: 1.3351x; 1.2077x over previous
"""Trainium2 Bass kernel for nn_Affinity (graph-matching affinity matrix).

Math per sample (validated against the reference):
  out[(a,c),(b,c')] = sum_{e2,e1} G2[a,e2] H2[b,e2] Me[e2,e1] G1[c,e1] H1[c,e1]
                      + diag(vec(Mp))

Key structural collapse (validated end-to-end in fp64 numpy):
  * The 1024x1024 output is a pure DOUBLE SCATTER of the 96x96 edge
    affinity matrix:  out[(a2[t],c1[e]), (b2[t],c'1[e])] = Me[t,e],
    plus diag(vec(Mp)).  (The reference's row-major flatten of Me pairs
    row-position t of the e1-enumeration with row-position t of the
    e2-enumeration; the placement below reproduces it exactly.)
  * Me = G1^T R + H1^T S with R|S = C1 G2 + C2 H2 | C2 G2 + C1 H2 and
    C_i = F1^T relu(l_i + l_i^T) F2 (32x32).  C1/C2 depend only on
    lambda/F, so all d=128 contractions run during the input-DMA window
    and concurrently with the rank chain; the adjacency-dependent path
    only does 32-contractions.

Device (1 sample per NeuronCore, fully static instruction stream):
  1. ONE packed input DMA (lambda | A | F | U); select constants
     generated on device during the DMA window (DVE + Pool split).
  2. Row-major edge ranks via masked prefix-scan; one-hot rank
     expansion; G/H + per-edge endpoints via accumulating matmuls.
  3. C-chain on PE/Act in parallel with the rank chain.
  4. ONE output DMA: [Me | c | c' | a | b | MpT] (96 x 132 f16).  The
     host unshard only places device-computed values at device-computed
     indices (the Kronecker one-hot scatter) and casts.
"""

import numpy as np

import concourse.bacc as bacc
import concourse.bass as bass
import concourse.mybir as mybir
import concourse.tile as tile
from concourse.bass_utils import run_bass_kernel_spmd

F32 = mybir.dt.float32
F16 = mybir.dt.float16
I32 = mybir.dt.int32
ALU = mybir.AluOpType
AX = mybir.AxisListType
AF = mybir.ActivationFunctionType

B, N, D, E = 8, 32, 128, 96
NCORES = 8
OUTW = 132


def build_program(debug: bool = False):
    nc = bacc.Bacc("TRN2", target_bir_lowering=False, debug=debug,
                   num_devices=NCORES)
    # inp: lambda1 | lambda2 | A(16) | F1 | F2 | U1 | U2   (128, 400) f16
    inp = nc.dram_tensor("inp", [128, 400], F16, kind="ExternalInput")
    out_me = nc.dram_tensor("out_me", [96, OUTW], F16, kind="ExternalOutput")

    with tile.TileContext(nc) as tc:
        with tc.tile_pool(name="sb", bufs=1) as sb, \
             tc.tile_pool(name="ps", bufs=1, space="PSUM") as ps:
            in_sb = sb.tile([128, 400], F16, tag="in_sb")
            nc.sync.dma_start(out=in_sb[:], in_=inp[:, :])
            l1_16, l2_16 = in_sb[:, 0:128], in_sb[:, 128:256]
            a16 = in_sb[:, 256:272]             # A_src cols 0:8, A_tgt 8:16
            f1, f2 = in_sb[:, 272:304], in_sb[:, 304:336]
            u1, u2 = in_sb[:, 336:368], in_sb[:, 368:400]

            # ---- on-device constants (input-DMA window) ----
            it32 = sb.tile([128, 129], I32, tag="it32")
            nc.gpsimd.iota(it32[:], pattern=[[1, 129]], base=0,
                           channel_multiplier=0)
            pi32 = sb.tile([128, 1], I32, tag="pi32")
            nc.gpsimd.iota(pi32[:], pattern=[[1, 1]], base=0,
                           channel_multiplier=1)
            io16 = sb.tile([128, 129], F16, tag="io16")
            nc.vector.tensor_copy(out=io16[:], in_=it32[:])
            io32 = sb.tile([128, 128], F32, tag="io32")
            nc.vector.tensor_copy(out=io32[:], in_=it32[:, 0:128])
            pf32 = sb.tile([128, 1], F32, tag="pf32")
            nc.vector.tensor_copy(out=pf32[:], in_=pi32[:])
            q32 = sb.tile([128, 1], I32, tag="q32")
            nc.vector.tensor_scalar(out=q32[:], in0=pi32[:], scalar1=2,
                                    scalar2=None,
                                    op0=ALU.logical_shift_right)
            qf32 = sb.tile([128, 1], F32, tag="qf32")
            nc.vector.tensor_copy(out=qf32[:], in_=q32[:])
            headsel = sb.tile([128, 32], F16, tag="headsel")
            nc.vector.tensor_scalar(out=headsel[:], in0=io16[:, 0:32],
                                    scalar1=qf32[:, 0:1], scalar2=None,
                                    op0=ALU.is_equal)
            m32 = sb.tile([128, 1], I32, tag="m32")
            nc.vector.tensor_scalar(out=m32[:], in0=pi32[:], scalar1=3,
                                    scalar2=None, op0=ALU.bitwise_and)
            b832 = sb.tile([128, 1], I32, tag="b832")
            nc.vector.tensor_scalar(out=b832[:], in0=m32[:], scalar1=3,
                                    scalar2=None, op0=ALU.logical_shift_left)
            b8f = sb.tile([128, 1], F32, tag="b8f")
            nc.vector.tensor_copy(out=b8f[:], in_=b832[:])
            # hv16[:, 2k] = head value p//4; hv16[:, 2k+1] = tail 8(p%4)+k
            hv16 = sb.tile([128, 16], F16, tag="hv16")
            for k in range(8):
                nc.vector.tensor_copy(out=hv16[:, 2 * k:2 * k + 1],
                                      in_=qf32[:])
                nc.vector.tensor_scalar(out=hv16[:, 2 * k + 1:2 * k + 2],
                                        in0=b8f[:], scalar1=float(k),
                                        scalar2=None, op0=ALU.add)
            # Pool-side bigger constants (after the iotas)
            id16 = sb.tile([128, 128], F16, tag="id16")
            nc.gpsimd.tensor_scalar(out=id16[:], in0=io16[:, 0:128],
                                    scalar1=pf32[:, 0:1], scalar2=None,
                                    op0=ALU.is_equal)
            su32 = sb.tile([128, 128], F32, tag="su32")
            nc.gpsimd.tensor_scalar(out=su32[:], in0=io32[:],
                                    scalar1=pf32[:, 0:1], scalar2=None,
                                    op0=ALU.is_gt)
            tailsel = sb.tile([128, 256], F16, tag="tailsel")
            for k in range(8):
                nc.gpsimd.tensor_scalar(out=tailsel[:, 32 * k:32 * (k + 1)],
                                        in0=io16[:, 0:32],
                                        scalar1=b8f[:, 0:1],
                                        scalar2=float(k),
                                        op0=ALU.subtract, op1=ALU.is_equal)

            # ================= C-chain (PE + Act) =================
            lamp = ps.tile([128, 256], F32, tag="psL", bufs=1)
            for i, l_ in enumerate((l1_16, l2_16)):
                nc.tensor.matmul(out=lamp[:, 128 * i:128 * (i + 1)],
                                 lhsT=id16[:], rhs=l_,
                                 start=True, stop=False)
                nc.tensor.matmul(out=lamp[:, 128 * i:128 * (i + 1)],
                                 lhsT=l_, rhs=id16[:],
                                 start=False, stop=True)
            lp16 = sb.tile([128, 256], F16, tag="lp16")
            nc.scalar.activation(out=lp16[:], in_=lamp[:], func=AF.Relu)
            bps = ps.tile([128, 64], F32, tag="psA", bufs=1)
            nc.tensor.matmul(out=bps[:, 0:32], lhsT=lp16[:, 0:128], rhs=f2,
                             start=True, stop=True)
            nc.tensor.matmul(out=bps[:, 32:64], lhsT=lp16[:, 128:256],
                             rhs=f2, start=True, stop=True)
            b16 = sb.tile([128, 64], F16, tag="b16")
            nc.scalar.copy(out=b16[:], in_=bps[:])
            dps = ps.tile([32, 64], F32, tag="psL", bufs=1)
            nc.tensor.matmul(out=dps[:, 0:32], lhsT=b16[:, 0:32], rhs=f1,
                             start=True, stop=True)
            nc.tensor.matmul(out=dps[:, 32:64], lhsT=b16[:, 32:64], rhs=f1,
                             start=True, stop=True)
            d16 = sb.tile([32, 64], F16, tag="d16")
            nc.scalar.copy(out=d16[:], in_=dps[:])

            # ================= rank chain =================
            maskb = sb.tile([128, 16], F32, tag="maskb")
            nc.vector.tensor_copy(out=maskb[:], in_=a16)
            s2 = sb.tile([128, 2], F32, tag="s2")
            m3 = maskb[:].rearrange("p (g k) -> p g k", k=8)
            nc.vector.tensor_reduce(out=s2[:], in_=m3, axis=AX.X, op=ALU.add)
            pb = ps.tile([128, 2], F32, tag="psI", bufs=1)
            nc.tensor.matmul(out=pb[:], lhsT=su32[:], rhs=s2[:],
                             start=True, stop=True)
            r0 = sb.tile([128, 16], F32, tag="r0")
            r1h = sb.tile([128, 16], F32, tag="r1h")
            for g in (1, 0):
                nc.vector.tensor_tensor_scan(
                    out=r0[:, 8 * g:8 * (g + 1)],
                    data0=maskb[:, 8 * g:8 * (g + 1)],
                    data1=maskb[:, 8 * g:8 * (g + 1)],
                    initial=pb[:, g:g + 1],
                    op0=ALU.add, op1=ALU.bypass)
                nc.vector.tensor_tensor(out=r1h[:, 8 * g:8 * (g + 1)],
                                        in0=r0[:, 8 * g:8 * (g + 1)],
                                        in1=maskb[:, 8 * g:8 * (g + 1)],
                                        op=ALU.mult)

            # ---- graph2: one-hots + G2/H2 + endpoint matmuls ----
            caps = ps.tile([96, 36], F32, tag="psE", bufs=1)
            oh2t = sb.tile([128, 768], F16, tag="oh2")
            gh2ps = ps.tile([32, 192], F32, tag="psG2", bufs=1)
            for k in range(8):
                nc.vector.tensor_scalar(
                    out=oh2t[:, 96 * k:96 * (k + 1)],
                    in0=io16[:, 1:97],
                    scalar1=r1h[:, 8 + k:9 + k], scalar2=None,
                    op0=ALU.is_equal)
            for k in range(8):
                nc.tensor.matmul(out=gh2ps[:, 0:96], lhsT=headsel[:],
                                 rhs=oh2t[:, 96 * k:96 * (k + 1)],
                                 start=(k == 0), stop=(k == 7))
            for k in range(8):
                nc.tensor.matmul(out=gh2ps[:, 96:192],
                                 lhsT=tailsel[:, 32 * k:32 * (k + 1)],
                                 rhs=oh2t[:, 96 * k:96 * (k + 1)],
                                 start=(k == 0), stop=(k == 7))
            for k in range(8):
                nc.tensor.matmul(out=caps[:, 2:4],
                                 lhsT=oh2t[:, 96 * k:96 * (k + 1)],
                                 rhs=hv16[:, 2 * k:2 * (k + 1)],
                                 start=(k == 0), stop=(k == 7))
            gh2t = sb.tile([32, 192], F16, tag="gh2sb")
            nc.scalar.copy(out=gh2t[:], in_=gh2ps[:])

            # ---- graph1: one-hots + G1/H1 + endpoint matmuls ----
            oh1t = sb.tile([128, 768], F16, tag="oh1")
            gh1ps = ps.tile([32, 192], F32, tag="psG1", bufs=1)
            for k in range(8):
                nc.vector.tensor_scalar(
                    out=oh1t[:, 96 * k:96 * (k + 1)],
                    in0=io16[:, 1:97],
                    scalar1=r1h[:, k:k + 1], scalar2=None,
                    op0=ALU.is_equal)
            for k in range(8):
                nc.tensor.matmul(out=gh1ps[:, 0:96], lhsT=headsel[:],
                                 rhs=oh1t[:, 96 * k:96 * (k + 1)],
                                 start=(k == 0), stop=(k == 7))
            for k in range(8):
                nc.tensor.matmul(out=gh1ps[:, 96:192],
                                 lhsT=tailsel[:, 32 * k:32 * (k + 1)],
                                 rhs=oh1t[:, 96 * k:96 * (k + 1)],
                                 start=(k == 0), stop=(k == 7))
            for k in range(8):
                nc.tensor.matmul(out=caps[:, 0:2],
                                 lhsT=oh1t[:, 96 * k:96 * (k + 1)],
                                 rhs=hv16[:, 2 * k:2 * (k + 1)],
                                 start=(k == 0), stop=(k == 7))
            nc.tensor.matmul(out=caps[0:32, 4:36], lhsT=u2, rhs=u1,
                             start=True, stop=True)
            gh1t = sb.tile([32, 192], F16, tag="gh1sb")
            nc.vector.tensor_copy(out=gh1t[:], in_=gh1ps[:])

            # ---- R|S = C1 G2 + C2 H2 | C2 G2 + C1 H2  (32, 192) ----
            rsps = ps.tile([32, 192], F32, tag="psA", bufs=1)
            g2sb, h2sb = gh2t[:, 0:96], gh2t[:, 96:192]
            nc.tensor.matmul(out=rsps[:, 0:96], lhsT=d16[:, 0:32], rhs=g2sb,
                             start=True, stop=False)
            nc.tensor.matmul(out=rsps[:, 0:96], lhsT=d16[:, 32:64], rhs=h2sb,
                             start=False, stop=True)
            nc.tensor.matmul(out=rsps[:, 96:192], lhsT=d16[:, 32:64],
                             rhs=g2sb, start=True, stop=False)
            nc.tensor.matmul(out=rsps[:, 96:192], lhsT=d16[:, 0:32],
                             rhs=h2sb, start=False, stop=True)
            rs16 = sb.tile([32, 192], F16, tag="rs16")
            nc.scalar.copy(out=rs16[:], in_=rsps[:])

            # ---- Me = G1^T R + H1^T S  (96, 96) ----
            meps = ps.tile([96, 96], F32, tag="psL", bufs=1)
            nc.tensor.matmul(out=meps[:], lhsT=gh1t[:, 0:96],
                             rhs=rs16[:, 0:96], start=True, stop=False)
            nc.tensor.matmul(out=meps[:], lhsT=gh1t[:, 96:192],
                             rhs=rs16[:, 96:192], start=False, stop=True)

            # ---- pack [Me | c | c' | a | b | MpT] and single DMA ----
            outsb = sb.tile([96, OUTW], F16, tag="outsb")
            nc.vector.tensor_copy(out=outsb[:, 96:OUTW], in_=caps[:])
            nc.vector.tensor_copy(out=outsb[:, 0:96], in_=meps[:])
            nc.sync.dma_start(out=out_me[:, :], in_=outsb[:, :])
    nc.compile()
    return nc


def make_in_maps(inputs: dict) -> list:
    inputs = {k: np.asarray(v, dtype=np.float32) for k, v in inputs.items()}
    in_maps = []
    for b in range(B):
        pk = np.zeros((128, 400), np.float16)
        pk[:, 0:128] = inputs["lambda1"]
        pk[:, 128:256] = inputs["lambda2"]
        pk[:, 256:264] = inputs["A_src"][b].reshape(128, 8)
        pk[:, 264:272] = inputs["A_tgt"][b].reshape(128, 8)
        pk[:, 272:304] = inputs["F_src"][b]
        pk[:, 304:336] = inputs["F_tgt"][b]
        pk[:, 336:368] = inputs["U_src"][b]
        pk[:, 368:400] = inputs["U_tgt"][b]
        in_maps.append({"inp": np.ascontiguousarray(pk)})
    return in_maps


_NC_CACHE = {}


def _assemble(res: dict) -> np.ndarray:
    """Place device-computed Me values at device-computed indices.

    out[(a2[t], c1[e]), (b2[t], c'1[e])] = Me[t, e]; out[i,i] += vec(Mp)[i].
    Pure placement + fp16->fp32 cast; no arithmetic on input data.
    """
    o = res["out_me"].astype(np.float32)
    me = o[:, 0:96]
    c = np.rint(o[:, 96]).astype(np.int64)
    cp = np.rint(o[:, 97]).astype(np.int64)
    a = np.rint(o[:, 98]).astype(np.int64)
    bb = np.rint(o[:, 99]).astype(np.int64)
    mpt = o[0:32, 100:132]                               # MpT[c, a]
    outm = np.zeros((1024, 1024), np.float32)
    o4 = outm.reshape(32, 32, 32, 32)
    o4[a[:, None], c[None, :], bb[:, None], cp[None, :]] = me
    outm[np.arange(1024), np.arange(1024)] += mpt.T.ravel()
    return outm


def kernel(trace: bool = False, **inputs) -> np.ndarray:
    if "nc" not in _NC_CACHE:
        _NC_CACHE["nc"] = build_program()
    nc = _NC_CACHE["nc"]
    in_maps = make_in_maps(inputs)
    res = run_bass_kernel_spmd(nc, in_maps, core_ids=list(range(NCORES)),
                               trace=trace)
    _NC_CACHE["last_results"] = res
    outs = [_assemble(res.results[b]) for b in range(B)]
    return np.stack(outs).astype(np.float32)


# revision 67
# speedup vs baseline: 1.5060x; 1.1280x over previous
"""Trainium2 Bass kernel for nn_Affinity (graph-matching affinity matrix).

Math per sample (validated against the reference):
  out[(a,c),(b,c')] = sum_{e2,e1} G2[a,e2] H2[b,e2] Me[e2,e1] G1[c,e1] H1[c,e1]
                      + diag(vec(Mp))

Key structural collapse (validated end-to-end in fp64 numpy):
  * The 1024x1024 output is a pure DOUBLE SCATTER of the 96x96 edge
    affinity matrix:  out[(a2[t],c1[e]), (b2[t],c'1[e])] = Me[t,e],
    plus diag(vec(Mp)).  (The reference's row-major flatten of Me pairs
    row-position t of the e1-enumeration with row-position t of the
    e2-enumeration; the placement below reproduces it exactly.)
  * Me = G1^T R + H1^T S with R|S = C1 G2 + C2 H2 | C2 G2 + C1 H2 and
    C_i = F1^T relu(l_i + l_i^T) F2 (32x32).  C1/C2 depend only on
    lambda/F, so all d=128 contractions run during the input-DMA window
    and concurrently with the rank chain; the adjacency-dependent path
    only does 32-contractions.

Device (1 sample per NeuronCore, fully static instruction stream):
  1. ONE packed input DMA (lambda | A | F | U); select constants
     generated on device during the DMA window (DVE + Pool split).
  2. Row-major edge ranks via masked prefix-scan; one-hot rank
     expansion; G/H + per-edge endpoints via accumulating matmuls.
  3. C-chain on PE/Act in parallel with the rank chain.
  4. ONE output DMA: [Me | c | c' | a | b | MpT] (96 x 132 f16).  The
     host unshard only places device-computed values at device-computed
     indices (the Kronecker one-hot scatter) and casts.
"""

import numpy as np

import concourse.bacc as bacc
import concourse.bass as bass
import concourse.mybir as mybir
import concourse.tile as tile
from concourse.bass_utils import run_bass_kernel_spmd

F32 = mybir.dt.float32
F16 = mybir.dt.float16
I32 = mybir.dt.int32
ALU = mybir.AluOpType
AX = mybir.AxisListType
AF = mybir.ActivationFunctionType

B, N, D, E = 8, 32, 128, 96
NCORES = 8
OUTW = 132


def build_program(debug: bool = False):
    nc = bacc.Bacc("TRN2", target_bir_lowering=False, debug=debug,
                   num_devices=NCORES)
    # inp: lambda1 | lambda2 | A(16) | F1 | F2 | U1 | U2   (128, 400) f16
    inp = nc.dram_tensor("inp", [128, 400], F16, kind="ExternalInput")
    out_me = nc.dram_tensor("out_me", [96, OUTW], F16, kind="ExternalOutput")

    with tile.TileContext(nc) as tc:
        with tc.tile_pool(name="sb", bufs=1) as sb, \
             tc.tile_pool(name="ps", bufs=1, space="PSUM") as ps:
            in_sb = sb.tile([128, 400], F16, tag="in_sb")
            nc.sync.dma_start(out=in_sb[:], in_=inp[:, :])
            l1_16, l2_16 = in_sb[:, 0:128], in_sb[:, 128:256]
            a16 = in_sb[:, 256:272]             # A_src cols 0:8, A_tgt 8:16
            f1, f2 = in_sb[:, 272:304], in_sb[:, 304:336]
            u1, u2 = in_sb[:, 336:368], in_sb[:, 368:400]

            # dummy activation: absorbs the 1283ns act-table load up front
            # so the scheduler doesn't model the lamp relu as late
            scr = sb.tile([128, 2], F16, tag="scr")
            nc.scalar.memzero(scr[:])
            nc.scalar.activation(out=scr[:], in_=scr[:], func=AF.Relu)

            # ---- on-device constants (input-DMA window) ----
            it32 = sb.tile([128, 129], I32, tag="it32")
            nc.gpsimd.iota(it32[:], pattern=[[1, 129]], base=0,
                           channel_multiplier=0)
            pi32 = sb.tile([128, 1], I32, tag="pi32")
            nc.gpsimd.iota(pi32[:], pattern=[[1, 1]], base=0,
                           channel_multiplier=1)
            io16 = sb.tile([128, 129], F16, tag="io16")
            nc.vector.tensor_copy(out=io16[:], in_=it32[:])
            pf32 = sb.tile([128, 1], F32, tag="pf32")
            nc.vector.tensor_copy(out=pf32[:], in_=pi32[:])
            q32 = sb.tile([128, 1], I32, tag="q32")
            nc.vector.tensor_scalar(out=q32[:], in0=pi32[:], scalar1=2,
                                    scalar2=None,
                                    op0=ALU.logical_shift_right)
            qf32 = sb.tile([128, 1], F32, tag="qf32")
            nc.vector.tensor_copy(out=qf32[:], in_=q32[:])
            m32 = sb.tile([128, 1], I32, tag="m32")
            nc.vector.tensor_scalar(out=m32[:], in0=pi32[:], scalar1=3,
                                    scalar2=None, op0=ALU.bitwise_and)
            b832 = sb.tile([128, 1], I32, tag="b832")
            nc.vector.tensor_scalar(out=b832[:], in0=m32[:], scalar1=3,
                                    scalar2=None, op0=ALU.logical_shift_left)
            b8f = sb.tile([128, 1], F32, tag="b8f")
            nc.vector.tensor_copy(out=b8f[:], in_=b832[:])
            # hv16[:, 2k] = head value p//4; hv16[:, 2k+1] = tail 8(p%4)+k
            hv16 = sb.tile([128, 16], F16, tag="hv16")
            for k in range(8):
                nc.vector.tensor_copy(out=hv16[:, 2 * k:2 * k + 1],
                                      in_=qf32[:])
                nc.vector.tensor_scalar(out=hv16[:, 2 * k + 1:2 * k + 2],
                                        in0=b8f[:], scalar1=float(k),
                                        scalar2=None, op0=ALU.add)
            # hts: per-k stacked select [headsel | tailsel_k]  (128, 8*64)
            # head blocks in ONE strided DVE op; tail blocks on Pool
            hts = sb.tile([128, 512], F16, tag="hts")
            hts3 = hts[:].rearrange("p (k c) -> p k c", c=64)
            io_b = io16[:, 0:32].unsqueeze(1).broadcast_to([128, 8, 32])
            nc.vector.tensor_scalar(out=hts3[:, :, 0:32], in0=io_b,
                                    scalar1=qf32[:, 0:1], scalar2=None,
                                    op0=ALU.is_equal)
            # Pool-side bigger constants (after the iotas)
            id16 = sb.tile([128, 128], F16, tag="id16")
            nc.gpsimd.tensor_scalar(out=id16[:], in0=io16[:, 0:128],
                                    scalar1=pf32[:, 0:1], scalar2=None,
                                    op0=ALU.is_equal)
            su16 = sb.tile([128, 128], F16, tag="su16")
            nc.gpsimd.tensor_scalar(out=su16[:], in0=io16[:, 0:128],
                                    scalar1=pf32[:, 0:1], scalar2=None,
                                    op0=ALU.is_gt)
            for k in range(8):
                nc.gpsimd.tensor_scalar(out=hts[:, 64 * k + 32:64 * (k + 1)],
                                        in0=io16[:, 0:32],
                                        scalar1=b8f[:, 0:1],
                                        scalar2=float(k),
                                        op0=ALU.subtract, op1=ALU.is_equal)

            # ================= C-chain (PE + Act) =================
            lamp = ps.tile([128, 256], F32, tag="psL", bufs=1)
            for i, l_ in enumerate((l1_16, l2_16)):
                nc.tensor.matmul(out=lamp[:, 128 * i:128 * (i + 1)],
                                 lhsT=id16[:], rhs=l_,
                                 start=True, stop=False)
                nc.tensor.matmul(out=lamp[:, 128 * i:128 * (i + 1)],
                                 lhsT=l_, rhs=id16[:],
                                 start=False, stop=True)
            lp16 = sb.tile([128, 256], F16, tag="lp16")
            nc.scalar.activation(out=lp16[:], in_=lamp[:], func=AF.Relu)
            # B in both column orders: [B1|B2|B2|B1] so lhsT slices give
            # stacked [D1;D2] and [D2;D1] with no partition offsets
            bps = ps.tile([128, 128], F32, tag="psA", bufs=1)
            nc.tensor.matmul(out=bps[:, 0:32], lhsT=lp16[:, 0:128], rhs=f2,
                             start=True, stop=True)
            nc.tensor.matmul(out=bps[:, 32:64], lhsT=lp16[:, 128:256],
                             rhs=f2, start=True, stop=True)
            nc.tensor.matmul(out=bps[:, 64:96], lhsT=lp16[:, 128:256],
                             rhs=f2, start=True, stop=True)
            nc.tensor.matmul(out=bps[:, 96:128], lhsT=lp16[:, 0:128],
                             rhs=f2, start=True, stop=True)
            b16 = sb.tile([128, 128], F16, tag="b16")
            nc.scalar.copy(out=b16[:], in_=bps[:])
            # dstack = [[D1;D2] | [D2;D1]]  (64, 64)
            dps = ps.tile([64, 64], F32, tag="psL", bufs=1)
            nc.tensor.matmul(out=dps[:, 0:32], lhsT=b16[:, 0:64], rhs=f1,
                             start=True, stop=True)
            nc.tensor.matmul(out=dps[:, 32:64], lhsT=b16[:, 64:128], rhs=f1,
                             start=True, stop=True)
            d16 = sb.tile([64, 64], F16, tag="d16")
            nc.vector.tensor_copy(out=d16[:], in_=dps[:])


            # ===== rank chain (f16 throughout; counts <= 96 are exact) =====
            s2 = sb.tile([128, 2], F16, tag="s2")
            m3 = a16.rearrange("p (g k) -> p g k", k=8)
            r0 = sb.tile([128, 16], F16, tag="r0")
            r1h = sb.tile([128, 16], F32, tag="r1h")  # is_equal scalar: f32
            pb = ps.tile([128, 2], F32, tag="psI", bufs=1)
            with nc.allow_low_precision(reason="integer counts <= 96"):
                nc.vector.tensor_reduce(out=s2[:], in_=m3, axis=AX.X,
                                        op=ALU.add)
                nc.tensor.matmul(out=pb[:], lhsT=su16[:], rhs=s2[:],
                                 start=True, stop=True)
                for g in (1, 0):
                    nc.vector.tensor_tensor_scan(
                        out=r0[:, 8 * g:8 * (g + 1)],
                        data0=a16[:, 8 * g:8 * (g + 1)],
                        data1=a16[:, 8 * g:8 * (g + 1)],
                        initial=pb[:, g:g + 1],
                        op0=ALU.add, op1=ALU.bypass)
                    nc.vector.tensor_tensor(out=r1h[:, 8 * g:8 * (g + 1)],
                                            in0=r0[:, 8 * g:8 * (g + 1)],
                                            in1=a16[:, 8 * g:8 * (g + 1)],
                                            op=ALU.mult)

            # ---- graph2: one-hots + stacked [G2;H2] + endpoint matmuls ----
            caps = ps.tile([96, 36], F32, tag="psE", bufs=1)
            oh2t = sb.tile([128, 768], F16, tag="oh2")
            gh2ps = ps.tile([64, 96], F32, tag="psG2", bufs=1)
            for k in range(8):
                nc.vector.tensor_scalar(
                    out=oh2t[:, 96 * k:96 * (k + 1)],
                    in0=io16[:, 1:97],
                    scalar1=r1h[:, 8 + k:9 + k], scalar2=None,
                    op0=ALU.is_equal)
            for k in range(8):
                nc.tensor.matmul(out=gh2ps[:], lhsT=hts[:, 64 * k:64 * (k + 1)],
                                 rhs=oh2t[:, 96 * k:96 * (k + 1)],
                                 start=(k == 0), stop=(k == 7))
            for k in range(8):
                nc.tensor.matmul(out=caps[:, 2:4],
                                 lhsT=oh2t[:, 96 * k:96 * (k + 1)],
                                 rhs=hv16[:, 2 * k:2 * (k + 1)],
                                 start=(k == 0), stop=(k == 7))
            gh2t = sb.tile([64, 96], F16, tag="gh2sb")
            nc.scalar.copy(out=gh2t[:], in_=gh2ps[:])

            # ---- graph1: one-hots (k5-7 on Pool) + stacked [G1;H1] ----
            oh1t = sb.tile([128, 768], F16, tag="oh1")
            gh1ps = ps.tile([64, 96], F32, tag="psG1", bufs=1)
            for k in range(8):
                eng = nc.vector if k < 5 else nc.gpsimd
                eng.tensor_scalar(
                    out=oh1t[:, 96 * k:96 * (k + 1)],
                    in0=io16[:, 1:97],
                    scalar1=r1h[:, k:k + 1], scalar2=None,
                    op0=ALU.is_equal)
            for k in range(8):
                nc.tensor.matmul(out=gh1ps[:],
                                 lhsT=hts[:, 64 * k:64 * (k + 1)],
                                 rhs=oh1t[:, 96 * k:96 * (k + 1)],
                                 start=(k == 0), stop=(k == 7))
            for k in range(8):
                nc.tensor.matmul(out=caps[:, 0:2],
                                 lhsT=oh1t[:, 96 * k:96 * (k + 1)],
                                 rhs=hv16[:, 2 * k:2 * (k + 1)],
                                 start=(k == 0), stop=(k == 7))
            nc.tensor.matmul(out=caps[0:32, 4:36], lhsT=u2, rhs=u1,
                             start=True, stop=True)
            gh1t = sb.tile([64, 96], F16, tag="gh1sb")
            nc.vector.tensor_copy(out=gh1t[:], in_=gh1ps[:])

            # ---- [R;S] = d16^T @ [G2;H2]: ONE matmul, no offsets ----
            rsps = ps.tile([64, 96], F32, tag="psA", bufs=1)
            nc.tensor.matmul(out=rsps[:], lhsT=d16[:], rhs=gh2t[:],
                             start=True, stop=True)
            rs16 = sb.tile([64, 96], F16, tag="rs16")
            nc.scalar.copy(out=rs16[:], in_=rsps[:])

            # ---- Me = [G1;H1]^T [R;S]  (96, 96): ONE matmul ----
            meps = ps.tile([96, 96], F32, tag="psL", bufs=1)
            nc.tensor.matmul(out=meps[:], lhsT=gh1t[:], rhs=rs16[:],
                             start=True, stop=True)

            # ---- pack [Me | c | c' | a | b | MpT] and single DMA ----
            outsb = sb.tile([96, OUTW], F16, tag="outsb")
            nc.vector.tensor_copy(out=outsb[:, 96:OUTW], in_=caps[:])
            nc.vector.tensor_copy(out=outsb[:, 0:96], in_=meps[:])
            nc.sync.dma_start(out=out_me[:, :], in_=outsb[:, :])
    nc.compile()
    return nc


def make_in_maps(inputs: dict) -> list:
    inputs = {k: np.asarray(v, dtype=np.float32) for k, v in inputs.items()}
    in_maps = []
    for b in range(B):
        pk = np.zeros((128, 400), np.float16)
        pk[:, 0:128] = inputs["lambda1"]
        pk[:, 128:256] = inputs["lambda2"]
        pk[:, 256:264] = inputs["A_src"][b].reshape(128, 8)
        pk[:, 264:272] = inputs["A_tgt"][b].reshape(128, 8)
        pk[:, 272:304] = inputs["F_src"][b]
        pk[:, 304:336] = inputs["F_tgt"][b]
        pk[:, 336:368] = inputs["U_src"][b]
        pk[:, 368:400] = inputs["U_tgt"][b]
        in_maps.append({"inp": np.ascontiguousarray(pk)})
    return in_maps


_NC_CACHE = {}


def _assemble(res: dict) -> np.ndarray:
    """Place device-computed Me values at device-computed indices.

    out[(a2[t], c1[e]), (b2[t], c'1[e])] = Me[t, e]; out[i,i] += vec(Mp)[i].
    Pure placement + fp16->fp32 cast; no arithmetic on input data.
    """
    o = res["out_me"].astype(np.float32)
    me = o[:, 0:96]
    c = np.rint(o[:, 96]).astype(np.int64)
    cp = np.rint(o[:, 97]).astype(np.int64)
    a = np.rint(o[:, 98]).astype(np.int64)
    bb = np.rint(o[:, 99]).astype(np.int64)
    mpt = o[0:32, 100:132]                               # MpT[c, a]
    outm = np.zeros((1024, 1024), np.float32)
    o4 = outm.reshape(32, 32, 32, 32)
    o4[a[:, None], c[None, :], bb[:, None], cp[None, :]] = me
    outm[np.arange(1024), np.arange(1024)] += mpt.T.ravel()
    return outm


def kernel(trace: bool = False, **inputs) -> np.ndarray:
    if "nc" not in _NC_CACHE:
        _NC_CACHE["nc"] = build_program()
    nc = _NC_CACHE["nc"]
    in_maps = make_in_maps(inputs)
    res = run_bass_kernel_spmd(nc, in_maps, core_ids=list(range(NCORES)),
                               trace=trace)
    _NC_CACHE["last_results"] = res
    outs = [_assemble(res.results[b]) for b in range(B)]
    return np.stack(outs).astype(np.float32)


# revision 68
# speedup vs baseline: 1.5196x; 1.0090x over previous
"""Trainium2 Bass kernel for nn_Affinity (graph-matching affinity matrix).

Math per sample (validated against the reference):
  out[(a,c),(b,c')] = sum_{e2,e1} G2[a,e2] H2[b,e2] Me[e2,e1] G1[c,e1] H1[c,e1]
                      + diag(vec(Mp))

Key structural collapse (validated end-to-end in fp64 numpy):
  * The 1024x1024 output is a pure DOUBLE SCATTER of the 96x96 edge
    affinity matrix:  out[(a2[t],c1[e]), (b2[t],c'1[e])] = Me[t,e],
    plus diag(vec(Mp)).  (The reference's row-major flatten of Me pairs
    row-position t of the e1-enumeration with row-position t of the
    e2-enumeration; the placement below reproduces it exactly.)
  * Me = G1^T R + H1^T S with R|S = C1 G2 + C2 H2 | C2 G2 + C1 H2 and
    C_i = F1^T relu(l_i + l_i^T) F2 (32x32).  C1/C2 depend only on
    lambda/F, so all d=128 contractions run during the input-DMA window
    and concurrently with the rank chain; the adjacency-dependent path
    only does 32-contractions.

Device (1 sample per NeuronCore, fully static instruction stream):
  1. ONE packed input DMA (lambda | A | F | U); select constants
     generated on device during the DMA window (DVE + Pool split).
  2. Row-major edge ranks via masked prefix-scan; one-hot rank
     expansion; G/H + per-edge endpoints via accumulating matmuls.
  3. C-chain on PE/Act in parallel with the rank chain.
  4. ONE output DMA: [Me | c | c' | a | b | MpT] (96 x 132 f16).  The
     host unshard only places device-computed values at device-computed
     indices (the Kronecker one-hot scatter) and casts.
"""

import numpy as np

import concourse.bacc as bacc
import concourse.bass as bass
import concourse.mybir as mybir
import concourse.tile as tile
from concourse.bass_utils import run_bass_kernel_spmd

F32 = mybir.dt.float32
F16 = mybir.dt.float16
I32 = mybir.dt.int32
ALU = mybir.AluOpType
AX = mybir.AxisListType
AF = mybir.ActivationFunctionType

B, N, D, E = 8, 32, 128, 96
NCORES = 8
OUTW = 132


def build_program(debug: bool = False):
    nc = bacc.Bacc("TRN2", target_bir_lowering=False, debug=debug,
                   num_devices=NCORES)
    # inp: lambda1 | lambda2 | A(16) | F1 | F2 | U1 | U2   (128, 400) f16
    inp = nc.dram_tensor("inp", [128, 400], F16, kind="ExternalInput")
    out_me = nc.dram_tensor("out_me", [96, OUTW], F16, kind="ExternalOutput")

    with tile.TileContext(nc) as tc:
        with tc.tile_pool(name="sb", bufs=1) as sb, \
             tc.tile_pool(name="ps", bufs=1, space="PSUM") as ps:
            in_sb = sb.tile([128, 400], F16, tag="in_sb")
            nc.sync.dma_start(out=in_sb[:, 0:272], in_=inp[:, 0:272])
            nc.sync.dma_start(out=in_sb[:, 272:400], in_=inp[:, 272:400])
            l1_16, l2_16 = in_sb[:, 0:128], in_sb[:, 128:256]
            a16 = in_sb[:, 256:272]             # A_src cols 0:8, A_tgt 8:16
            f1, f2 = in_sb[:, 272:304], in_sb[:, 304:336]
            u1, u2 = in_sb[:, 336:368], in_sb[:, 368:400]

            # dummy activation: absorbs the 1283ns act-table load up front
            # so the scheduler doesn't model the lamp relu as late
            scr = sb.tile([128, 2], F16, tag="scr")
            nc.scalar.memzero(scr[:])
            nc.scalar.activation(out=scr[:], in_=scr[:], func=AF.Relu)

            # ---- on-device constants (input-DMA window) ----
            it32 = sb.tile([128, 129], I32, tag="it32")
            nc.gpsimd.iota(it32[:], pattern=[[1, 129]], base=0,
                           channel_multiplier=0)
            pi32 = sb.tile([128, 1], I32, tag="pi32")
            nc.gpsimd.iota(pi32[:], pattern=[[1, 1]], base=0,
                           channel_multiplier=1)
            io16 = sb.tile([128, 129], F16, tag="io16")
            nc.vector.tensor_copy(out=io16[:], in_=it32[:])
            pf32 = sb.tile([128, 1], F32, tag="pf32")
            nc.vector.tensor_copy(out=pf32[:], in_=pi32[:])
            q32 = sb.tile([128, 1], I32, tag="q32")
            nc.vector.tensor_scalar(out=q32[:], in0=pi32[:], scalar1=2,
                                    scalar2=None,
                                    op0=ALU.logical_shift_right)
            qf32 = sb.tile([128, 1], F32, tag="qf32")
            nc.vector.tensor_copy(out=qf32[:], in_=q32[:])
            m32 = sb.tile([128, 1], I32, tag="m32")
            nc.vector.tensor_scalar(out=m32[:], in0=pi32[:], scalar1=3,
                                    scalar2=None, op0=ALU.bitwise_and)
            b832 = sb.tile([128, 1], I32, tag="b832")
            nc.vector.tensor_scalar(out=b832[:], in0=m32[:], scalar1=3,
                                    scalar2=None, op0=ALU.logical_shift_left)
            b8f = sb.tile([128, 1], F32, tag="b8f")
            nc.vector.tensor_copy(out=b8f[:], in_=b832[:])
            # hv16[:, 2k] = head value p//4; hv16[:, 2k+1] = tail 8(p%4)+k
            hv16 = sb.tile([128, 16], F16, tag="hv16")
            for k in range(8):
                nc.vector.tensor_copy(out=hv16[:, 2 * k:2 * k + 1],
                                      in_=qf32[:])
                nc.vector.tensor_scalar(out=hv16[:, 2 * k + 1:2 * k + 2],
                                        in0=b8f[:], scalar1=float(k),
                                        scalar2=None, op0=ALU.add)
            # hts: per-k stacked select [headsel | tailsel_k]  (128, 8*64)
            # head blocks in ONE strided DVE op; tail blocks on Pool
            hts = sb.tile([128, 512], F16, tag="hts")
            hts3 = hts[:].rearrange("p (k c) -> p k c", c=64)
            io_b = io16[:, 0:32].unsqueeze(1).broadcast_to([128, 8, 32])
            nc.vector.tensor_scalar(out=hts3[:, :, 0:32], in0=io_b,
                                    scalar1=qf32[:, 0:1], scalar2=None,
                                    op0=ALU.is_equal)
            # Pool-side bigger constants (after the iotas)
            id16 = sb.tile([128, 128], F16, tag="id16")
            nc.gpsimd.tensor_scalar(out=id16[:], in0=io16[:, 0:128],
                                    scalar1=pf32[:, 0:1], scalar2=None,
                                    op0=ALU.is_equal)
            su16 = sb.tile([128, 128], F16, tag="su16")
            nc.gpsimd.tensor_scalar(out=su16[:], in0=io16[:, 0:128],
                                    scalar1=pf32[:, 0:1], scalar2=None,
                                    op0=ALU.is_gt)
            for k in range(8):
                nc.gpsimd.tensor_scalar(out=hts[:, 64 * k + 32:64 * (k + 1)],
                                        in0=io16[:, 0:32],
                                        scalar1=b8f[:, 0:1],
                                        scalar2=float(k),
                                        op0=ALU.subtract, op1=ALU.is_equal)

            # ================= C-chain (PE + Act) =================
            lamp = ps.tile([128, 256], F32, tag="psL", bufs=1)
            for i, l_ in enumerate((l1_16, l2_16)):
                nc.tensor.matmul(out=lamp[:, 128 * i:128 * (i + 1)],
                                 lhsT=id16[:], rhs=l_,
                                 start=True, stop=False)
                nc.tensor.matmul(out=lamp[:, 128 * i:128 * (i + 1)],
                                 lhsT=l_, rhs=id16[:],
                                 start=False, stop=True)
            lp16 = sb.tile([128, 256], F16, tag="lp16")
            nc.scalar.activation(out=lp16[:], in_=lamp[:], func=AF.Relu)
            # B in both column orders: [B1|B2|B2|B1] so lhsT slices give
            # stacked [D1;D2] and [D2;D1] with no partition offsets
            bps = ps.tile([128, 128], F32, tag="psA", bufs=1)
            nc.tensor.matmul(out=bps[:, 0:32], lhsT=lp16[:, 0:128], rhs=f2,
                             start=True, stop=True)
            nc.tensor.matmul(out=bps[:, 32:64], lhsT=lp16[:, 128:256],
                             rhs=f2, start=True, stop=True)
            nc.tensor.matmul(out=bps[:, 64:96], lhsT=lp16[:, 128:256],
                             rhs=f2, start=True, stop=True)
            nc.tensor.matmul(out=bps[:, 96:128], lhsT=lp16[:, 0:128],
                             rhs=f2, start=True, stop=True)
            b16 = sb.tile([128, 128], F16, tag="b16")
            nc.scalar.copy(out=b16[:], in_=bps[:])
            # dstack = [[D1;D2] | [D2;D1]]  (64, 64)
            dps = ps.tile([64, 64], F32, tag="psL", bufs=1)
            nc.tensor.matmul(out=dps[:, 0:32], lhsT=b16[:, 0:64], rhs=f1,
                             start=True, stop=True)
            nc.tensor.matmul(out=dps[:, 32:64], lhsT=b16[:, 64:128], rhs=f1,
                             start=True, stop=True)
            d16 = sb.tile([64, 64], F16, tag="d16")
            nc.vector.tensor_copy(out=d16[:], in_=dps[:])


            # ===== rank chain (f16 throughout; counts <= 96 are exact) =====
            s2 = sb.tile([128, 2], F16, tag="s2")
            m3 = a16.rearrange("p (g k) -> p g k", k=8)
            r0 = sb.tile([128, 16], F16, tag="r0")
            r1h = sb.tile([128, 16], F32, tag="r1h")  # is_equal scalar: f32
            pb = ps.tile([128, 2], F32, tag="psI", bufs=1)
            with nc.allow_low_precision(reason="integer counts <= 96"):
                nc.vector.tensor_reduce(out=s2[:], in_=m3, axis=AX.X,
                                        op=ALU.add)
                nc.tensor.matmul(out=pb[:], lhsT=su16[:], rhs=s2[:],
                                 start=True, stop=True)
                for g in (1, 0):
                    nc.vector.tensor_tensor_scan(
                        out=r0[:, 8 * g:8 * (g + 1)],
                        data0=a16[:, 8 * g:8 * (g + 1)],
                        data1=a16[:, 8 * g:8 * (g + 1)],
                        initial=pb[:, g:g + 1],
                        op0=ALU.add, op1=ALU.bypass)
                    nc.vector.tensor_tensor(out=r1h[:, 8 * g:8 * (g + 1)],
                                            in0=r0[:, 8 * g:8 * (g + 1)],
                                            in1=a16[:, 8 * g:8 * (g + 1)],
                                            op=ALU.mult)

            # ---- graph2: one-hots + stacked [G2;H2] + endpoint matmuls ----
            caps = ps.tile([96, 36], F32, tag="psE", bufs=1)
            oh2t = sb.tile([128, 768], F16, tag="oh2")
            gh2ps = ps.tile([64, 96], F32, tag="psG2", bufs=1)
            for k in range(8):
                nc.vector.tensor_scalar(
                    out=oh2t[:, 96 * k:96 * (k + 1)],
                    in0=io16[:, 1:97],
                    scalar1=r1h[:, 8 + k:9 + k], scalar2=None,
                    op0=ALU.is_equal)
            for k in range(8):
                nc.tensor.matmul(out=gh2ps[:], lhsT=hts[:, 64 * k:64 * (k + 1)],
                                 rhs=oh2t[:, 96 * k:96 * (k + 1)],
                                 start=(k == 0), stop=(k == 7))
            for k in range(8):
                nc.tensor.matmul(out=caps[:, 2:4],
                                 lhsT=oh2t[:, 96 * k:96 * (k + 1)],
                                 rhs=hv16[:, 2 * k:2 * (k + 1)],
                                 start=(k == 0), stop=(k == 7))
            gh2t = sb.tile([64, 96], F16, tag="gh2sb")
            nc.scalar.copy(out=gh2t[:], in_=gh2ps[:])

            # ---- graph1: one-hots (k5-7 on Pool) + stacked [G1;H1] ----
            oh1t = sb.tile([128, 768], F16, tag="oh1")
            gh1ps = ps.tile([64, 96], F32, tag="psG1", bufs=1)
            for k in range(8):
                eng = nc.vector if k < 5 else nc.gpsimd
                eng.tensor_scalar(
                    out=oh1t[:, 96 * k:96 * (k + 1)],
                    in0=io16[:, 1:97],
                    scalar1=r1h[:, k:k + 1], scalar2=None,
                    op0=ALU.is_equal)
            for k in range(8):
                nc.tensor.matmul(out=gh1ps[:],
                                 lhsT=hts[:, 64 * k:64 * (k + 1)],
                                 rhs=oh1t[:, 96 * k:96 * (k + 1)],
                                 start=(k == 0), stop=(k == 7))
            for k in range(8):
                nc.tensor.matmul(out=caps[:, 0:2],
                                 lhsT=oh1t[:, 96 * k:96 * (k + 1)],
                                 rhs=hv16[:, 2 * k:2 * (k + 1)],
                                 start=(k == 0), stop=(k == 7))
            nc.tensor.matmul(out=caps[0:32, 4:36], lhsT=u2, rhs=u1,
                             start=True, stop=True)
            gh1t = sb.tile([64, 96], F16, tag="gh1sb")
            nc.vector.tensor_copy(out=gh1t[:], in_=gh1ps[:])

            # ---- [R;S] = d16^T @ [G2;H2]: ONE matmul, no offsets ----
            rsps = ps.tile([64, 96], F32, tag="psA", bufs=1)
            nc.tensor.matmul(out=rsps[:], lhsT=d16[:], rhs=gh2t[:],
                             start=True, stop=True)
            rs16 = sb.tile([64, 96], F16, tag="rs16")
            nc.scalar.copy(out=rs16[:], in_=rsps[:])

            # ---- Me = [G1;H1]^T [R;S]  (96, 96): ONE matmul ----
            meps = ps.tile([96, 96], F32, tag="psL", bufs=1)
            nc.tensor.matmul(out=meps[:], lhsT=gh1t[:], rhs=rs16[:],
                             start=True, stop=True)

            # ---- pack [Me | c | c' | a | b | MpT] and single DMA ----
            outsb = sb.tile([96, OUTW], F16, tag="outsb")
            nc.vector.tensor_copy(out=outsb[:, 96:OUTW], in_=caps[:])
            nc.vector.tensor_copy(out=outsb[:, 0:96], in_=meps[:])
            nc.sync.dma_start(out=out_me[:, :], in_=outsb[:, :])
    nc.compile()
    return nc


def make_in_maps(inputs: dict) -> list:
    inputs = {k: np.asarray(v, dtype=np.float32) for k, v in inputs.items()}
    in_maps = []
    for b in range(B):
        pk = np.zeros((128, 400), np.float16)
        pk[:, 0:128] = inputs["lambda1"]
        pk[:, 128:256] = inputs["lambda2"]
        pk[:, 256:264] = inputs["A_src"][b].reshape(128, 8)
        pk[:, 264:272] = inputs["A_tgt"][b].reshape(128, 8)
        pk[:, 272:304] = inputs["F_src"][b]
        pk[:, 304:336] = inputs["F_tgt"][b]
        pk[:, 336:368] = inputs["U_src"][b]
        pk[:, 368:400] = inputs["U_tgt"][b]
        in_maps.append({"inp": np.ascontiguousarray(pk)})
    return in_maps


_NC_CACHE = {}


def _assemble(res: dict) -> np.ndarray:
    """Place device-computed Me values at device-computed indices.

    out[(a2[t], c1[e]), (b2[t], c'1[e])] = Me[t, e]; out[i,i] += vec(Mp)[i].
    Pure placement + fp16->fp32 cast; no arithmetic on input data.
    """
    o = res["out_me"].astype(np.float32)
    me = o[:, 0:96]
    c = np.rint(o[:, 96]).astype(np.int64)
    cp = np.rint(o[:, 97]).astype(np.int64)
    a = np.rint(o[:, 98]).astype(np.int64)
    bb = np.rint(o[:, 99]).astype(np.int64)
    mpt = o[0:32, 100:132]                               # MpT[c, a]
    outm = np.zeros((1024, 1024), np.float32)
    o4 = outm.reshape(32, 32, 32, 32)
    o4[a[:, None], c[None, :], bb[:, None], cp[None, :]] = me
    outm[np.arange(1024), np.arange(1024)] += mpt.T.ravel()
    return outm


def kernel(trace: bool = False, **inputs) -> np.ndarray:
    if "nc" not in _NC_CACHE:
        _NC_CACHE["nc"] = build_program()
    nc = _NC_CACHE["nc"]
    in_maps = make_in_maps(inputs)
    res = run_bass_kernel_spmd(nc, in_maps, core_ids=list(range(NCORES)),
                               trace=trace)
    _NC_CACHE["last_results"] = res
    outs = [_assemble(res.results[b]) for b in range(B)]
    return np.stack(outs).astype(np.float32)
